# revision 1
# baseline (speedup 1.0000x reference)
"""Distributed Trainium2 Bass kernel for the contextual-attention module.

Strategy (per sharding hint): data-parallel over batch (2 samples x 4 cores),
within a sample the L=4096 patch/kernel axis is sharded 4 ways (1024 kernels
per core = 16 rows of patch centers).  Per core:

  scores[l, s]  = sum_{c,dy,dx} kern_bf[l,c,dy,dx] * boxfeat_bf[c, y+dy, x+dx]
  (the reference's 3x3 box-sum of scores is commuted into a 3x3 box filter
   of the feature map, so it rides along in the same GEMM)
  kernel L2 normalization is folded in as a per-l row scale (rnorm) applied
  to scores (pre-softmax) and to attn (pre-transpose-conv).
  softmax over the full L axis is flash-style: exp against the LOCAL max,
  then one 4-core AllGather of (max, sum) stat rows per spatial chunk and a
  local combine - the collective stays off the PE critical path.
  transpose-conv: per (dy,dx), partial[c, s] = kern^T @ attn accumulated in
  PSUM, overlap-added into a padded canvas; final blend
  out = canvas*(1-mask)/9 + feat*mask/4 (the /4 makes the feat term sum to
  1x across the group) followed by a 4-core ReduceScatter over channels.

Each core returns a [32, 4096] channel band; the host stitches the full
[2, 128, 64, 64] output.
"""

import os
import sys
import types

for _p in ("/opt/trn_rl_repo",):
    if os.path.isdir(_p) and _p not in sys.path:
        sys.path.append(_p)


def _ensure_axon_hooks():
    """Make antenv.axon_hooks importable so bass_utils trace mode never
    crashes on the import (hook may still be None -> tracing is skipped)."""
    try:
        import antenv.axon_hooks  # noqa: F401
        return
    except Exception:
        pass
    try:
        import antenv
        mod = types.ModuleType("antenv.axon_hooks")
        mod._hook = None

        def set_axon_ntff_profile_hook(hook):
            mod._hook = hook

        def get_axon_ntff_profile_hook():
            return mod._hook

        mod.set_axon_ntff_profile_hook = set_axon_ntff_profile_hook
        mod.get_axon_ntff_profile_hook = get_axon_ntff_profile_hook
        sys.modules["antenv.axon_hooks"] = mod
        antenv.axon_hooks = mod
    except Exception:
        pass


_ensure_axon_hooks()

import numpy as np  # noqa: E402

NCH = 128           # channels
W = H = 64          # spatial
S = W * H           # 4096 spatial positions
B = 2               # batch
G = 4               # cores per sample
NCORES = 8
LS = S // G         # kernels per core (1024)
LT = LS // 128      # l-tiles per core (8)
ROWS = 8            # patch-center rows per chunk
CS = ROWS * H       # spatial chunk (512)
NCHUNK = W // ROWS  # 8 chunks
EPS = 1e-7

_CACHE = {}
LAST_EXEC_TIME_NS = None


def _build():
    from concourse import bacc, tile, mybir
    from concourse.masks import make_identity

    F32 = mybir.dt.float32
    FR = mybir.dt.float32r
    BF = mybir.dt.bfloat16
    Alu = mybir.AluOpType
    Act = mybir.ActivationFunctionType
    AxC = mybir.AxisListType.C
    AxX = mybir.AxisListType.X

    nc = bacc.Bacc("TRN2", target_bir_lowering=False, debug=False,
                   num_devices=NCORES)

    fg_ext = nc.dram_tensor("fg", [NCH, S], F32, kind="ExternalInput")
    fgband_ext = nc.dram_tensor("fgband", [NCH, 18 * H], F32,
                                kind="ExternalInput")
    mask_ext = nc.dram_tensor("mask", [1, S], F32, kind="ExternalInput")
    mband_ext = nc.dram_tensor("maskband", [1, 18 * H], F32,
                               kind="ExternalInput")
    out_ext = nc.dram_tensor("out", [NCH // G, S], F32, kind="ExternalOutput")

    groups = [[0, 1, 2, 3], [4, 5, 6, 7]]

    with tile.TileContext(nc) as tc:
        with tc.tile_pool(name="const", bufs=1) as cpool, \
             tc.tile_pool(name="pers", bufs=1) as pers, \
             tc.tile_pool(name="big", bufs=1) as big, \
             tc.tile_pool(name="psA", bufs=4, space="PSUM") as psA, \
             tc.tile_pool(name="psT", bufs=2, space="PSUM") as psT, \
             tc.tile_pool(name="psS", bufs=2, space="PSUM") as psS, \
             tc.tile_pool(name="dram", bufs=3, space="DRAM") as dram, \
             tc.tile_pool(name="dramP", bufs=1, space="DRAM") as dramP:

            ident_b = cpool.tile([128, 128], BF, tag="idb")
            make_identity(nc, ident_b[:])
            ones_cb = cpool.tile([128, 1], BF, tag="ones")
            nc.gpsimd.memset(ones_cb[:], 1.0)

            # ---------------- persistent tensors ----------------
            boxbf = pers.tile([NCH, 66, 66], BF, tag="boxbf")
            kernT = pers.tile([NCH, 9, LS], BF, tag="kernT")
            kern_lc = pers.tile([128, 9, LT, NCH], BF, tag="kernlc")

            Q_CHUNKS = [(0, 2), (2, 2), (4, 3), (7, 1)]
            canvas_q = [
                dramP.tile([NCH, n * CS], F32, tag=f"cin{h}", name=f"cin{h}")
                for h, (s0, n) in enumerate(Q_CHUNKS)]
            rs_q = [
                dramP.tile([NCH // G, n * CS], F32, tag=f"rso{h}",
                           name=f"rso{h}")
                for h, (s0, n) in enumerate(Q_CHUNKS)]

            bar_in = dramP.tile([16], F32, tag="bari")
            bar_out = dramP.tile([16 * NCORES], F32, tag="baro")
            bar2_in = dramP.tile([16], F32, tag="bari2")
            bar2_out = dramP.tile([16 * G], F32, tag="baro2")

            with tc.tile_pool(name="prep", bufs=1) as prep:
                # ------------ prep: box-filtered feature map ------------
                fgtmp = big.tile([NCH, W, H], F32, tag="big66")
                nc.sync.dma_start(
                    fgtmp[:], fg_ext[:].rearrange("c (y x) -> c y x", y=W))
                hp = prep.tile([NCH, W, 63], BF, tag="hvp")
                nc.vector.tensor_add(hp[:, 0:18, :], fgtmp[:, 0:18, 0:63],
                                     fgtmp[:, 0:18, 1:64])
                tmpH = prep.tile([NCH, W, 66], BF, tag="tmpH")
                nc.vector.tensor_add(tmpH[:, 0:18, 2:64], hp[:, 0:18, 0:62],
                                     fgtmp[:, 0:18, 2:64])
                nc.vector.tensor_add(hp[:, 18:64, :], fgtmp[:, 18:64, 0:63],
                                     fgtmp[:, 18:64, 1:64])
                nc.vector.tensor_add(tmpH[:, 18:64, 2:64], hp[:, 18:64, 0:62],
                                     fgtmp[:, 18:64, 2:64])
                nc.vector.tensor_copy(tmpH[:, :, 0:1], fgtmp[:, :, 0:1])
                nc.vector.tensor_copy(tmpH[:, :, 1:2], hp[:, :, 0:1])
                nc.vector.tensor_copy(tmpH[:, :, 64:65], hp[:, :, 62:63])
                nc.vector.tensor_copy(tmpH[:, :, 65:66], fgtmp[:, :, 63:64])
                vp = prep.tile([NCH, 63, 66], BF, tag="hvp")
                nc.vector.tensor_add(vp[:, 0:17, :], tmpH[:, 0:17, :],
                                     tmpH[:, 1:18, :])
                nc.vector.tensor_add(boxbf[:, 2:18, :], vp[:, 0:16, :],
                                     tmpH[:, 2:18, :])
                nc.vector.tensor_copy(boxbf[:, 0:1, :], tmpH[:, 0:1, :])
                nc.vector.tensor_copy(boxbf[:, 1:2, :], vp[:, 0:1, :])
                nc.vector.tensor_add(vp[:, 17:63, :], tmpH[:, 17:63, :],
                                     tmpH[:, 18:64, :])
                nc.vector.tensor_add(boxbf[:, 18:64, :], vp[:, 16:62, :],
                                     tmpH[:, 18:64, :])
                nc.vector.tensor_copy(boxbf[:, 64:65, :], vp[:, 62:63, :])
                nc.vector.tensor_copy(boxbf[:, 65:66, :], tmpH[:, 63:64, :])

                # ------------ prep: kernels ------------
                fgband_sb = prep.tile([NCH, 18, H], F32, tag="fgband")
                nc.sync.dma_start(
                    fgband_sb[:],
                    fgband_ext[:].rearrange("c (r x) -> c r x", r=18))
                mband_row = prep.tile([1, 18 * H], F32, tag="mbandrow")
                nc.sync.dma_start(mband_row[:], mband_ext[:])
                # warm-up barrier: depends on loaded input, syncs the cores
                # and absorbs ncfw cold-start before the first real AllGather
                nc.gpsimd.dma_start(bar_in[:], fgband_sb[0:1, 0, 0:16])
                nc.gpsimd.collective_compute(
                    "AllGather", Alu.bypass,
                    replica_groups=[list(range(NCORES))],
                    ins=[bar_in.opt()], outs=[bar_out.opt()])
                # warm the 4-core-group communicator path the stats
                # AllGathers use (the 8-core barrier doesn't cover it)
                nc.gpsimd.dma_start(bar2_in[:], fgband_sb[0:1, 0, 16:32])
                nc.gpsimd.collective_compute(
                    "AllGather", Alu.bypass, replica_groups=groups,
                    ins=[bar2_in.opt()], outs=[bar2_out.opt()])
                mband_bc = prep.tile([NCH, 18 * H], F32, tag="mbandbc")
                nc.gpsimd.partition_broadcast(mband_bc[:], mband_row[:])
                bgbandp = prep.tile([NCH, 18, 66], F32, tag="bgbandp")
                nc.gpsimd.memset(bgbandp[:], 0.0)
                nc.vector.tensor_mul(
                    bgbandp[:, :, 1:65], fgband_sb[:],
                    mband_bc[:].rearrange("c (r x) -> c r x", r=18))
                for d in range(9):
                    dy, dx = d // 3, d % 3
                    nc.vector.tensor_scalar_add(
                        kernT[:, d, :],
                        bgbandp[:, dy:dy + 16, dx:dx + 64], EPS)

                # kernel norms: sumsq over (c, dydx) via ones-matmul, per l
                ps_s0 = psS.tile([1, 512], F32, tag="psS")
                ps_s1 = psS.tile([1, 512], F32, tag="psS")
                for d in range(9):
                    ksq0 = prep.tile([NCH, 512], BF, tag="ksq0")
                    ksq1 = prep.tile([NCH, 512], BF, tag="ksq1")
                    nc.scalar.activation(ksq0[:], kernT[:, d, 0:512],
                                         Act.Square)
                    nc.scalar.activation(ksq1[:], kernT[:, d, 512:1024],
                                         Act.Square)
                    nc.tensor.matmul(ps_s0[:], ones_cb[:], ksq0[:],
                                     start=(d == 0), stop=(d == 8))
                    nc.tensor.matmul(ps_s1[:], ones_cb[:], ksq1[:],
                                     start=(d == 0), stop=(d == 8))
                rnorm_row = prep.tile([1, LS], F32, tag="rnormrow")
                norm_row = prep.tile([1, LS], F32, tag="normrow")
                nc.scalar.activation(norm_row[:, 0:512], ps_s0[:], Act.Sqrt)
                nc.scalar.activation(norm_row[:, 512:1024], ps_s1[:],
                                     Act.Sqrt)
                # broadcast the norm, then reciprocal in [128, LS] layout
                norm_bc = prep.tile([NCH, LS], F32, tag="normbc")
                nc.gpsimd.partition_broadcast(norm_bc[:], norm_row[:])
                rnorm_bc = prep.tile([NCH, LS], F32, tag="rnormbc")
                nc.vector.reciprocal(rnorm_bc[:], norm_bc[:])
                for d in range(9):
                    eng = nc.vector if d < 5 else nc.gpsimd
                    eng.tensor_mul(kernT[:, d, :], kernT[:, d, :],
                                   rnorm_bc[:])

            canvas = big.tile([NCH, 66, 66], F32, tag="big66")
            nc.gpsimd.memset(canvas[:], 0.0)

            ctx2 = tc.tile_pool(name="chunk", bufs=2)
            wk = ctx2.__enter__()
            ctx2b = tc.tile_pool(name="chunk3", bufs=3)
            wk3 = ctx2b.__enter__()
            ctx3 = tc.tile_pool(name="stat", bufs=2)
            st = ctx3.__enter__()
            ctx3b = tc.tile_pool(name="stat3", bufs=3)
            st3 = ctx3b.__enter__()
            ctx4 = tc.tile_pool(name="blend", bufs=2)
            bl = ctx4.__enter__()

            # ---------------- pipelined chunk loop ----------------
            def emit_gemm1(k):
                r0 = k * ROWS
                scs = []
                mtmp = wk3.tile([128, CS], BF, tag="mtmp")
                for t in range(LT):
                    ps = psA.tile([128, CS], F32, tag="psA")
                    for d in range(9):
                        dy, dx = d // 3, d % 3
                        nc.tensor.matmul(
                            ps[:],
                            kernT[:, d, t * 128:(t + 1) * 128],
                            boxbf[:, r0 + dy:r0 + dy + ROWS, dx:dx + 64],
                            start=(d == 0), stop=(d == 8))
                    sc = wk3.tile([128, CS], F32, tag=f"sc{t}")
                    nc.scalar.activation(sc[:], ps[:], Act.Identity)
                    if t == 0:
                        nc.vector.tensor_copy(mtmp[:], ps[:])
                    else:
                        nc.vector.scalar_tensor_tensor(
                            mtmp[:], ps[:], 1.0, mtmp[:],
                            op0=Alu.mult, op1=Alu.max)
                    scs.append(sc)
                return scs, mtmp

            def emit_kern_lc():
                for d in range(9):
                    for t in range(LT):
                        pt = psT.tile([128, 128], BF, tag="psT")
                        nc.tensor.transpose(
                            pt[:], kernT[:, d, t * 128:(t + 1) * 128],
                            ident_b[:])
                        nc.vector.tensor_copy(kern_lc[:, d, t, :], pt[:])

            def emit_maxpath(k, mtmp):
                """local max of scores over l -> m_bc broadcast + AG input."""
                NT = CS // 128
                m_loc = st3.tile([128, NT], F32, tag="mloc")
                for j in range(NT):
                    pt = psT.tile([128, 128], BF, tag="psT")
                    # strided column set {col : col % NT == j} so that the
                    # p-major dump of m_loc is in natural column order
                    nc.tensor.transpose(
                        pt[:], mtmp[:, j::NT], ident_b[:])
                    nc.vector.tensor_reduce(m_loc[:, j:j + 1], pt[:], AxX,
                                            Alu.max)
                ag_in = dram.tile([2 * CS], F32, tag="agi")
                nc.sync.dma_start(ag_in[0:CS], m_loc[:])
                m_row = st3.tile([1, CS], F32, tag="mrow")
                nc.sync.dma_start(m_row[:], ag_in[0:CS])
                m_bc = st3.tile([128, CS], F32, tag="mbc")
                nc.gpsimd.partition_broadcast(m_bc[:], m_row[:])
                return ag_in, m_loc, m_bc

            def emit_subexp(k, scs, m_bc):
                ets = []
                for t in range(LT):
                    et = wk.tile([128, CS], BF, tag=f"et{t}")
                    nc.vector.tensor_sub(et[:], scs[t][:], m_bc[:])
                    nc.scalar.activation(et[:], et[:], Act.Exp)
                    ets.append(et)
                return ets

            def emit_sum_ag(k, ets, ag_in):
                ps_sum = psS.tile([1, CS], F32, tag="psS")
                for t in range(LT):
                    nc.tensor.matmul(ps_sum[:], ones_cb[:], ets[t][:],
                                     start=(t == 0), stop=(t == LT - 1))
                s_row = st.tile([1, CS], F32, tag="srow")
                nc.scalar.activation(s_row[:], ps_sum[:], Act.Identity)
                nc.sync.dma_start(ag_in[CS:2 * CS], s_row[:])
                ag_out = dram.tile([2 * CS * G], F32, tag="ago")
                nc.gpsimd.collective_compute(
                    "AllGather", Alu.bypass, replica_groups=groups,
                    ins=[ag_in.opt()], outs=[ag_out.opt()])
                return ag_out

            def emit_combine(k, ets, ag_in, ag_out):
                """combine gathered stats -> per-s softmax factor broadcast.
                Stats tiles use a [32, 16] layout (same linear col order as
                the dumps) to keep DMA descriptor counts low."""
                cm = st.tile([32, G, CS // 32], F32, tag="cm")
                cs = st.tile([32, G, CS // 32], F32, tag="cs")
                for r in range(G):
                    nc.sync.dma_start(cm[:, r, :],
                                      ag_out[r * 2 * CS:r * 2 * CS + CS])
                    nc.sync.dma_start(cs[:, r, :],
                                      ag_out[r * 2 * CS + CS:(r + 1) * 2 * CS])
                m32 = st.tile([32, CS // 32], F32, tag="m32")
                nc.sync.dma_start(m32[:], ag_in[0:CS])
                Mx = st.tile([32, CS // 32], F32, tag="Mx")
                nc.vector.tensor_reduce(
                    Mx[:], cm[:].rearrange("p r t -> p t r"), AxX, Alu.max)
                for r in range(G):
                    nc.vector.tensor_sub(cm[:, r, :], cm[:, r, :], Mx[:])
                nc.scalar.activation(cm[:], cm[:], Act.Exp)
                nc.vector.tensor_mul(cs[:], cs[:], cm[:])
                gs = st.tile([32, CS // 32], F32, tag="gs")
                nc.vector.tensor_reduce(
                    gs[:], cs[:].rearrange("p r t -> p t r"), AxX, Alu.add)
                rg = st.tile([32, CS // 32], F32, tag="rg")
                nc.vector.reciprocal(rg[:], gs[:])
                fac_sl = st.tile([32, CS // 32], F32, tag="facsl")
                nc.vector.tensor_sub(fac_sl[:], m32[:], Mx[:])
                nc.scalar.activation(fac_sl[:], fac_sl[:], Act.Exp)
                nc.vector.tensor_mul(fac_sl[:], fac_sl[:], rg[:])
                fac_dram = dram.tile([CS], F32, tag="facd")
                nc.sync.dma_start(fac_dram[:], fac_sl[:])
                fac_row = st.tile([1, CS], F32, tag="facrow")
                nc.sync.dma_start(fac_row[:], fac_dram[:])
                fac_bc = st.tile([128, CS], F32, tag="facbc")
                nc.gpsimd.partition_broadcast(fac_bc[:], fac_row[:])
                return fac_bc

            def emit_gemm2(k, ets, fac_bc):
                r0 = k * ROWS
                for d in range(9):
                    dy, dx = d // 3, d % 3
                    ps2 = psA.tile([128, CS], F32, tag="psA")
                    for t in range(LT):
                        nc.tensor.matmul(
                            ps2[:], kern_lc[:, d, t, :], ets[t][:],
                            start=(t == 0), stop=(t == LT - 1))
                    sca = wk.tile([128, CS], F32, tag="sca")
                    nc.vector.tensor_mul(sca[:], ps2[:], fac_bc[:])
                    csl = canvas[:, r0 + dy:r0 + dy + ROWS, dx:dx + 64]
                    nc.vector.tensor_add(
                        csl, csl,
                        sca[:].rearrange("p (r x) -> p r x", r=ROWS))

            # software pipeline, combine/GEMM2 delayed one iteration so the
            # AllGather gets a full period of latency slack:
            #  iter k: maxpath(k) | combine+attn(k-1) | GEMM2(k-1) |
            #          GEMM1(k+1) | sub/exp(k) | sum+AG(k)
            def emit_blend(k):
                r0 = k * ROWS
                cint = canvas[:, 1 + r0:1 + r0 + ROWS, 1:65]
                mrow = bl.tile([1, CS], F32, tag="mrow")
                nc.sync.dma_start(mrow[:], mask_ext[:, k * CS:(k + 1) * CS])
                mbc = bl.tile([128, CS], F32, tag="mbcb")
                nc.gpsimd.partition_broadcast(mbc[:], mrow[:])
                fgc = bl.tile([NCH, CS], F32, tag="fgc")
                nc.sync.dma_start(fgc[:], fg_ext[:, k * CS:(k + 1) * CS])
                mc = bl.tile([128, CS], F32, tag="mc")
                mc3 = mc[:].rearrange("p (r x) -> p r x", r=ROWS)
                nc.vector.tensor_mul(
                    mc3, cint, mbc[:].rearrange("p (r x) -> p r x", r=ROWS))
                nc.vector.tensor_sub(mc3, cint, mc3)
                mf = bl.tile([128, CS], F32, tag="mf")
                nc.vector.scalar_tensor_tensor(
                    mf[:], fgc[:], 1.0 / G, mbc[:], op0=Alu.mult,
                    op1=Alu.mult)
                outb = bl.tile([128, CS], F32, tag="mc")
                nc.vector.scalar_tensor_tensor(
                    outb[:], mc[:], 1.0 / 9.0, mf[:], op0=Alu.mult,
                    op1=Alu.add)
                h = next(i for i, (s0, n) in enumerate(Q_CHUNKS)
                         if s0 <= k < s0 + n)
                kk = k - Q_CHUNKS[h][0]
                nc.sync.dma_start(
                    canvas_q[h][:, kk * CS:(kk + 1) * CS], outb[:])

            def emit_rs_q(h):
                s0, n = Q_CHUNKS[h]
                nc.gpsimd.collective_compute(
                    "ReduceScatter", Alu.add, replica_groups=groups,
                    ins=[canvas_q[h].opt()], outs=[rs_q[h].opt()])
                nc.sync.dma_start(
                    out_ext[:, s0 * CS:(s0 + n) * CS], rs_q[h][:])

            g1 = {0: emit_gemm1(0)}
            emit_kern_lc()
            g1[1] = emit_gemm1(1)
            mp = {0: emit_maxpath(0, g1[0][1])}
            hist = {}
            for k in range(NCHUNK):
                scs_cur = g1[k][0]
                if k >= 1:
                    # sums of chunk k-1 run dense here (exps are long done)
                    pag_in, _, pets = hist[k - 1]
                    pag_out = emit_sum_ag(k - 1, pets, pag_in)
                    hist[k - 1] = (pag_in, pag_out, pets)
                if k == NCHUNK - 1:
                    # last chunk: run its softmax + AllGather immediately so
                    # only combine+GEMM2+blend+RS remain after the loop
                    ag_in, m_loc, m_bc = mp[k]
                    ets = emit_subexp(k, scs_cur, m_bc)
                    ag_out = emit_sum_ag(k, ets, ag_in)
                    hist[k] = (ag_in, ag_out, ets)
                if k + 1 < NCHUNK:
                    mp[k + 1] = emit_maxpath(k + 1, g1[k + 1][1])
                if k + 2 < NCHUNK:
                    g1[k + 2] = emit_gemm1(k + 2)
                if k >= 1:
                    pag_in, pag_out, pets = hist[k - 1]
                    fac_bc = emit_combine(k - 1, pets, pag_in, pag_out)
                    emit_gemm2(k - 1, pets, fac_bc)
                if k >= 2:
                    emit_blend(k - 2)
                if k in (4, 6):
                    emit_rs_q((k - 4) // 2)
                if k < NCHUNK - 1:
                    ag_in, m_loc, m_bc = mp[k]
                    ets = emit_subexp(k, scs_cur, m_bc)
                    hist[k] = (ag_in, None, ets)
                hist.pop(k - 2, None)
                g1.pop(k, None)
                mp.pop(k, None)
            pag_in, pag_out, pets = hist[NCHUNK - 1]
            fac_bc = emit_combine(NCHUNK - 1, pets, pag_in, pag_out)
            emit_gemm2(NCHUNK - 1, pets, fac_bc)
            emit_blend(NCHUNK - 2)
            emit_rs_q(2)
            emit_blend(NCHUNK - 1)
            emit_rs_q(3)

            # blend chunks are emitted inside the pipeline (emit_blend)


            ctx4.__exit__(None, None, None)
            ctx3b.__exit__(None, None, None)
            ctx3.__exit__(None, None, None)
            ctx2b.__exit__(None, None, None)
            ctx2.__exit__(None, None, None)

    nc.compile()
    return nc


def _shard_inputs(fg, mk):
    """fg [2,128,64,64] f32, mk [2,1,64,64] f32 -> per-core input maps."""
    in_maps = []
    for core in range(NCORES):
        b, r = core // G, core % G
        y0 = r * (W // G)
        feat = np.ascontiguousarray(fg[b].reshape(NCH, S), np.float32)
        mask = np.ascontiguousarray(mk[b].reshape(1, S), np.float32)
        band = np.zeros((NCH, 18, H), np.float32)
        mband = np.zeros((1, 18, H), np.float32)
        lo = y0 - 1
        src_lo = max(0, lo)
        src_hi = min(W, y0 + 17)
        band[:, src_lo - lo:src_hi - lo] = fg[b][:, src_lo:src_hi]
        mband[:, src_lo - lo:src_hi - lo] = mk[b][:, src_lo:src_hi]
        in_maps.append({
            "fg": feat,
            "fgband": np.ascontiguousarray(band.reshape(NCH, 18 * H)),
            "mask": mask,
            "maskband": np.ascontiguousarray(mband.reshape(1, 18 * H)),
        })
    return in_maps


def kernel(foreground, masks):
    global LAST_EXEC_TIME_NS
    from concourse.bass_utils import run_bass_kernel_spmd

    fg = np.asarray(foreground, np.float32)
    mk = np.asarray(masks, np.float32)
    assert fg.shape == (B, NCH, W, H) and mk.shape == (B, 1, W, H)

    nc = _CACHE.get("nc")
    if nc is None:
        nc = _build()
        _CACHE["nc"] = nc

    in_maps = _shard_inputs(fg, mk)
    trace = bool(os.environ.get("BASS_KERNEL_TRACE"))
    res = run_bass_kernel_spmd(nc, in_maps, core_ids=list(range(NCORES)),
                               trace=trace)
    LAST_EXEC_TIME_NS = res.exec_time_ns
    if res.exec_time_ns is not None:
        print(f"HW exec time: {res.exec_time_ns} ns")

    out = np.empty((B, NCH, W, H), np.float32)
    for core in range(NCORES):
        b, r = core // G, core % G
        out[b, 32 * r:32 * (r + 1)] = (
            res.results[core]["out"].reshape(32, W, H))
    return out



# revision 5
# speedup vs baseline: 1.1543x; 1.1543x over previous
"""Distributed Trainium2 Bass kernel for the contextual-attention module.

Sharding (per hint): data-parallel over batch (2 samples x 4 cores); within a
sample the L=4096 kernel axis is sharded 4 ways (1024 kernels / 16 patch-center
rows per core).  Per core, the 64x64 spatial grid is processed in 8 chunks of
8 rows (CS=512 positions):

  GEMM1  scores[l, s] = sum_{c,d} kern[l,c,d] * boxfeat[c, s+d]   (the 3x3
         box-sum of scores is commuted onto the feature map).  Kernel L2
         normalization rides as a per-partition activation scale on the
         PSUM->SBUF copy (rnorm[l]).
  softmax over the full L axis is flash-style: local max via one gpsimd
         partition_all_reduce, exp against it, local sums via ones-matmuls;
         (max, sum) stat rows for chunk pairs go through one 4-core
         AllGather and a local combine with ~2 pipeline periods of slack.
  GEMM2  is output-pixel-major: exp values live in per-l-tile padded attn
         maps ahalo[t] [128, 66, 66]; for each output chunk all 9 shifted
         windows x 8 l-tiles (72 matmuls, kernel index flipped, rnorm folded
         into kern_lc) accumulate into ONE PSUM tile -- no canvas overlap-add.
  blend  out = psum * (fac * (1-mask)/9) + feat*mask/4, the per-s factor
         broadcast once; per chunk-pair ReduceScatter over channels.

The PE instruction stream (GEMM2(k-2), GEMM1(k+1), sums(k) per iteration)
never waits on a collective, keeping the HAM clock gate warm.
"""

import os
import sys
import types

for _p in ("/opt/trn_rl_repo",):
    if os.path.isdir(_p) and _p not in sys.path:
        sys.path.append(_p)


def _ensure_axon_hooks():
    """Make antenv.axon_hooks importable so bass_utils trace mode never
    crashes on the import (hook may still be None -> tracing is skipped)."""
    try:
        import antenv.axon_hooks  # noqa: F401
        return
    except Exception:
        pass
    try:
        import antenv
        mod = types.ModuleType("antenv.axon_hooks")
        mod._hook = None

        def set_axon_ntff_profile_hook(hook):
            mod._hook = hook

        def get_axon_ntff_profile_hook():
            return mod._hook

        mod.set_axon_ntff_profile_hook = set_axon_ntff_profile_hook
        mod.get_axon_ntff_profile_hook = get_axon_ntff_profile_hook
        sys.modules["antenv.axon_hooks"] = mod
        antenv.axon_hooks = mod
    except Exception:
        pass


_ensure_axon_hooks()

import numpy as np  # noqa: E402

NCH = 128           # channels
W = H = 64          # spatial
S = W * H           # 4096 spatial positions
B = 2               # batch
G = 4               # cores per sample
NCORES = 8
LS = S // G         # kernels per core (1024)
LT = LS // 128      # l-tiles per core (8)
ROWS = 8            # patch-center rows per chunk
CS = ROWS * H       # spatial chunk (512)
NCHUNK = W // ROWS  # 8 chunks
EPS = 1e-7

_CACHE = {}
LAST_EXEC_TIME_NS = None


def _build():
    from concourse import bacc, bass_isa, tile, mybir
    from concourse.masks import make_identity

    F32 = mybir.dt.float32
    BF = mybir.dt.bfloat16
    Alu = mybir.AluOpType
    Act = mybir.ActivationFunctionType
    AxX = mybir.AxisListType.X
    RMax = bass_isa.ReduceOp.max

    nc = bacc.Bacc("TRN2", target_bir_lowering=False, debug=False,
                   num_devices=NCORES)

    fg_ext = nc.dram_tensor("fg", [NCH, S], F32, kind="ExternalInput")
    fgband_ext = nc.dram_tensor("fgband", [NCH, 18 * H], F32,
                                kind="ExternalInput")
    mask_ext = nc.dram_tensor("mask", [1, S], F32, kind="ExternalInput")
    mband_ext = nc.dram_tensor("maskband", [1, 18 * H], F32,
                               kind="ExternalInput")
    out_ext = nc.dram_tensor("out", [NCH // G, S], F32, kind="ExternalOutput")

    groups = [[0, 1, 2, 3], [4, 5, 6, 7]]

    with tile.TileContext(nc) as tc:
        with tc.tile_pool(name="const", bufs=1) as cpool, \
             tc.tile_pool(name="pers", bufs=1) as pers, \
             tc.tile_pool(name="psA", bufs=2, space="PSUM") as psA, \
             tc.tile_pool(name="psB", bufs=2, space="PSUM") as psB, \
             tc.tile_pool(name="psS", bufs=2, space="PSUM") as psS, \
             tc.tile_pool(name="dram", bufs=2, space="DRAM") as dram, \
             tc.tile_pool(name="dramP", bufs=1, space="DRAM") as dramP:

            ident_b = cpool.tile([128, 128], BF, tag="idb")
            make_identity(nc, ident_b[:])
            ones_cb = cpool.tile([128, 1], BF, tag="ones")
            nc.gpsimd.memset(ones_cb[:], 1.0)

            # ---------------- persistent tensors ----------------
            boxbf = pers.tile([NCH, 66, 66], BF, tag="boxbf")
            kernT = pers.tile([NCH, 9, LS], BF, tag="kernT")
            kern_lc = pers.tile([128, 9, LT, NCH], BF, tag="kernlc")
            ahalo = [pers.tile([128, 66, 66], BF, tag=f"ah{t}",
                               name=f"ah{t}")
                     for t in range(LT)]
            q32 = pers.tile([32, NCHUNK, CS // 32], F32, tag="q32")
            rnorm_col = pers.tile([128, LT], F32, tag="rnorm")

            bar_in = dramP.tile([16], F32, tag="bari")
            bar_out = dramP.tile([16 * NCORES], F32, tag="baro")
            bar2_in = dramP.tile([4 * CS], F32, tag="bari2")
            bar2_out = dramP.tile([4 * CS * G], F32, tag="baro2")
            nrm_dram = dramP.tile([LS], F32, tag="nrmd")

            with tc.tile_pool(name="prep", bufs=1) as prep:
                # ---- input loads ----
                mband_row = prep.tile([1, 18 * H], F32, tag="mbandrow")
                nc.sync.dma_start(mband_row[:], mband_ext[:])
                fgband_sb = prep.tile([NCH, 18, H], F32, tag="fgband")
                nc.sync.dma_start(
                    fgband_sb[:],
                    fgband_ext[:].rearrange("c (r x) -> c r x", r=18))
                fg_sb = prep.tile([NCH, W, H], F32, tag="fgsb")
                nc.sync.dma_start(
                    fg_sb[:], fg_ext[:].rearrange("c (y x) -> c y x", y=W))
                m32a = prep.tile([32, NCHUNK, CS // 32], F32, tag="m32a")
                for k in range(NCHUNK):
                    nc.sync.dma_start(m32a[:, k, :],
                                      mask_ext[:, k * CS:(k + 1) * CS])

                # ---- warmup collectives (absorb cold-start early) ----
                nc.gpsimd.dma_start(bar_in[:], mband_row[0:1, 0:16])
                nc.gpsimd.collective_compute(
                    "AllGather", Alu.bypass,
                    replica_groups=[list(range(NCORES))],
                    ins=[bar_in.opt()], outs=[bar_out.opt()])
                # warm the 4-core communicator with the REAL stats-AG size
                junk = prep.tile([32, 64], F32, tag="junk")
                nc.gpsimd.memset(junk[:], 0.0)
                nc.gpsimd.dma_start(bar2_in[:], junk[:])
                nc.gpsimd.collective_compute(
                    "AllGather", Alu.bypass, replica_groups=groups,
                    ins=[bar2_in.opt()], outs=[bar2_out.opt()])

                # ---- kernels: kernT[c, d, l] = (band*mask)[shifted] + EPS --
                mband_bc = prep.tile([NCH, 18 * H], BF, tag="mbandbc")
                mband_bf = prep.tile([1, 18 * H], BF, tag="mbandbf")
                nc.scalar.activation(mband_bf[:], mband_row[:], Act.Identity)
                nc.gpsimd.partition_broadcast(mband_bc[:], mband_bf[:])
                bgbandp = prep.tile([NCH, 18, 66], F32, tag="bgbandp")
                nc.gpsimd.memset(bgbandp[:], 0.0)
                nc.vector.tensor_mul(
                    bgbandp[:, :, 1:65], fgband_sb[:],
                    mband_bc[:].rearrange("c (r x) -> c r x", r=18))
                for d in range(9):
                    dy, dx = d // 3, d % 3
                    nc.vector.tensor_scalar_add(
                        kernT[:, d, :],
                        bgbandp[:, dy:dy + 16, dx:dx + 64], EPS)

                # ---- kernel norms: sumsq via squares + ones-matmul ----
                ksq = prep.tile([NCH, LS], BF, tag="ksq")
                ps_s0 = psS.tile([1, 512], F32, tag="psS")
                ps_s1 = psS.tile([1, 512], F32, tag="psS")
                for d in range(9):
                    nc.vector.tensor_mul(ksq[:], kernT[:, d, :],
                                         kernT[:, d, :])
                    nc.tensor.matmul(ps_s0[:], ones_cb[:], ksq[:, 0:512],
                                     start=(d == 0), stop=(d == 8))
                    nc.tensor.matmul(ps_s1[:], ones_cb[:], ksq[:, 512:1024],
                                     start=(d == 0), stop=(d == 8))
                srow = prep.tile([1, LS], F32, tag="srow")
                nc.scalar.activation(srow[:, 0:512], ps_s0[:], Act.Identity)
                nc.scalar.activation(srow[:, 512:1024], ps_s1[:],
                                     Act.Identity)
                # flatten [1, 1024] -> [128, 8] (partition p, col t <-
                # l = t*128 + p) via a DRAM bounce
                nc.gpsimd.dma_start(nrm_dram[:], srow[:])
                sq128 = prep.tile([128, LT], F32, tag="sq128")
                nc.gpsimd.dma_start(
                    sq128[:],
                    nrm_dram[:].rearrange("(t p) -> p t", p=128))
                norm128 = prep.tile([128, LT], F32, tag="norm128")
                nc.scalar.activation(norm128[:], sq128[:], Act.Sqrt)
                nc.vector.reciprocal(rnorm_col[:], norm128[:])

                # ---- blend constant: q32 = (1-mask)/9 in stat layout ----
                nc.vector.tensor_scalar(q32[:], m32a[:], -1.0 / 9.0,
                                        1.0 / 9.0, op0=Alu.mult, op1=Alu.add)

                # ---- box-filtered feature map (halo'ed, bf16) ----
                fgb = prep.tile([NCH, W, H], BF, tag="fgb")
                nc.scalar.activation(fgb[:], fg_sb[:], Act.Identity)
                hp = prep.tile([NCH, W, 63], BF, tag="hvp")
                nc.vector.tensor_add(hp[:], fgb[:, :, 0:63], fgb[:, :, 1:64])
                tmpH = prep.tile([NCH, W, 66], BF, tag="tmpH")
                nc.vector.tensor_add(tmpH[:, :, 2:64], hp[:, :, 0:62],
                                     fgb[:, :, 2:64])
                nc.vector.tensor_copy(tmpH[:, :, 0:1], fgb[:, :, 0:1])
                nc.vector.tensor_copy(tmpH[:, :, 1:2], hp[:, :, 0:1])
                nc.vector.tensor_copy(tmpH[:, :, 64:65], hp[:, :, 62:63])
                nc.vector.tensor_copy(tmpH[:, :, 65:66], fgb[:, :, 63:64])
                vp = prep.tile([NCH, 63, 66], BF, tag="hvp")
                nc.vector.tensor_add(vp[:], tmpH[:, 0:63, :],
                                     tmpH[:, 1:64, :])
                nc.vector.tensor_add(boxbf[:, 2:64, :], vp[:, 0:62, :],
                                     tmpH[:, 2:64, :])
                nc.vector.tensor_copy(boxbf[:, 0:1, :], tmpH[:, 0:1, :])
                nc.vector.tensor_copy(boxbf[:, 1:2, :], vp[:, 0:1, :])
                nc.vector.tensor_copy(boxbf[:, 64:65, :], vp[:, 62:63, :])
                nc.vector.tensor_copy(boxbf[:, 65:66, :], tmpH[:, 63:64, :])

                # ---- zero the attn maps (borders must stay 0) ----
                for t in range(LT):
                    nc.gpsimd.memset(ahalo[t][:], 0.0)

            # ---------------- chunk-loop pools ----------------
            ctx_scs = tc.tile_pool(name="scs", bufs=2)
            scsp = ctx_scs.__enter__()
            ctx_st = tc.tile_pool(name="stat", bufs=2)
            st = ctx_st.__enter__()
            ctx_bl = tc.tile_pool(name="blend", bufs=2)
            bl = ctx_bl.__enter__()

            def emit_gemm1(k):
                """scores for chunk k -> scs (normalized) + mtmp (max)."""
                r0 = k * ROWS
                scs = scsp.tile([128, LT, CS], F32, tag="scs")
                mtmp = st.tile([128, CS], F32, tag="mtmp")
                for t in range(LT):
                    ps = psA.tile([128, CS], F32, tag="psA")
                    for d in range(9):
                        dy, dx = d // 3, d % 3
                        nc.tensor.matmul(
                            ps[:],
                            kernT[:, d, t * 128:(t + 1) * 128],
                            boxbf[:, r0 + dy:r0 + dy + ROWS, dx:dx + 64],
                            start=(d == 0), stop=(d == 8))
                    nc.scalar.activation(scs[:, t, :], ps[:], Act.Identity,
                                         scale=rnorm_col[:, t:t + 1])
                    if t == 0:
                        nc.vector.tensor_copy(mtmp[:], scs[:, 0, :])
                    else:
                        nc.vector.scalar_tensor_tensor(
                            mtmp[:], scs[:, t, :], 1.0, mtmp[:],
                            op0=Alu.mult, op1=Alu.max)
                return scs, mtmp

            def emit_maxpath(k, mtmp):
                m_bc = st.tile([128, CS], F32, tag="mbc")
                nc.gpsimd.partition_all_reduce(m_bc[:], mtmp[:], 128, RMax)
                return m_bc

            def emit_subexp(k, scs, m_bc):
                r0 = k * ROWS
                for t in range(LT):
                    diff = st.tile([128, CS], F32, tag="diff")
                    nc.vector.tensor_sub(diff[:], scs[:, t, :], m_bc[:])
                    nc.scalar.activation(
                        ahalo[t][:, 1 + r0:9 + r0, 1:65],
                        diff[:].rearrange("p (r x) -> p r x", r=ROWS),
                        Act.Exp)

            def emit_sums(k, ag_in, m_bc):
                r0 = k * ROWS
                slot = k % 2
                ps_sum = psS.tile([1, CS], F32, tag="psS")
                for t in range(LT):
                    nc.tensor.matmul(
                        ps_sum[:], ones_cb[:],
                        ahalo[t][:, 1 + r0:9 + r0, 1:65],
                        start=(t == 0), stop=(t == LT - 1))
                s_row = st.tile([1, CS], F32, tag="srowc")
                nc.scalar.activation(s_row[:], ps_sum[:], Act.Identity)
                nc.gpsimd.dma_start(
                    ag_in[slot * 2 * CS:slot * 2 * CS + CS], m_bc[0:1, :])
                nc.gpsimd.dma_start(
                    ag_in[slot * 2 * CS + CS:(slot + 1) * 2 * CS], s_row[:])

            def emit_ag(pair):
                ag_out = dram.tile([4 * CS * G], F32, tag="ago")
                nc.gpsimd.collective_compute(
                    "AllGather", Alu.bypass, replica_groups=groups,
                    ins=[pair["in"].opt()], outs=[ag_out.opt()])
                pair["out"] = ag_out

            def emit_combine(k, pair):
                """gathered stats -> w_bc = fac * (1-mask)/9 broadcast.
                [32, 16] stat layout (linear col order, like the dumps)."""
                slot = k % 2
                ag_in, ag_out = pair["in"], pair["out"]
                cm = st.tile([32, G, CS // 32], F32, tag="cm")
                cs = st.tile([32, G, CS // 32], F32, tag="cs")
                for r in range(G):
                    base = r * 4 * CS + slot * 2 * CS
                    nc.gpsimd.dma_start(cm[:, r, :], ag_out[base:base + CS])
                    nc.gpsimd.dma_start(cs[:, r, :],
                                        ag_out[base + CS:base + 2 * CS])
                m32 = st.tile([32, CS // 32], F32, tag="m32")
                nc.gpsimd.dma_start(
                    m32[:], ag_in[slot * 2 * CS:slot * 2 * CS + CS])
                Mx = st.tile([32, CS // 32], F32, tag="Mx")
                nc.vector.tensor_reduce(
                    Mx[:], cm[:].rearrange("p r t -> p t r"), AxX, Alu.max)
                for r in range(G):
                    nc.vector.tensor_sub(cm[:, r, :], cm[:, r, :], Mx[:])
                nc.scalar.activation(cm[:], cm[:], Act.Exp)
                nc.vector.tensor_mul(cs[:], cs[:], cm[:])
                gs = st.tile([32, CS // 32], F32, tag="gs")
                nc.vector.tensor_reduce(
                    gs[:], cs[:].rearrange("p r t -> p t r"), AxX, Alu.add)
                rg = st.tile([32, CS // 32], F32, tag="rg")
                nc.vector.reciprocal(rg[:], gs[:])
                w_sl = st.tile([32, CS // 32], F32, tag="wsl")
                nc.vector.tensor_sub(w_sl[:], m32[:], Mx[:])
                nc.scalar.activation(w_sl[:], w_sl[:], Act.Exp)
                nc.vector.tensor_mul(w_sl[:], w_sl[:], rg[:])
                nc.vector.tensor_mul(w_sl[:], w_sl[:], q32[:, k, :])
                w_dram = dram.tile([CS], F32, tag="wd")
                nc.gpsimd.dma_start(w_dram[:], w_sl[:])
                w_row = st.tile([1, CS], F32, tag="wrow")
                nc.gpsimd.dma_start(w_row[:], w_dram[:])
                w_bc = st.tile([128, CS], F32, tag="wbc")
                nc.gpsimd.partition_broadcast(w_bc[:], w_row[:])
                return w_bc

            def emit_blend_prefetch(k):
                mrowk = bl.tile([1, CS], F32, tag="mrowk")
                nc.sync.dma_start(mrowk[:], mask_ext[:, k * CS:(k + 1) * CS])
                mbc = bl.tile([128, CS], F32, tag="mbck")
                nc.gpsimd.partition_broadcast(mbc[:], mrowk[:])
                fgc = bl.tile([NCH, CS], F32, tag="fgc")
                nc.sync.dma_start(fgc[:], fg_ext[:, k * CS:(k + 1) * CS])
                return mbc, fgc

            def emit_gemm2(k):
                r0 = k * ROWS
                ps2 = psB.tile([128, CS], F32, tag="psB")
                n = 0
                for dyp in range(3):
                    for dxp in range(3):
                        dflip = (2 - dyp) * 3 + (2 - dxp)
                        for t in range(LT):
                            nc.tensor.matmul(
                                ps2[:],
                                kern_lc[:, dflip, t, :],
                                ahalo[t][:, r0 + dyp:r0 + dyp + ROWS,
                                         dxp:dxp + 64],
                                start=(n == 0), stop=(n == 71))
                            n += 1
                return ps2

            def emit_blend(k, ps2, w_bc, mbc, fgc, pair_rs):
                slot = k % 2
                mfk = bl.tile([128, CS], F32, tag="mfk")
                nc.vector.scalar_tensor_tensor(
                    mfk[:], fgc[:], 1.0 / G, mbc[:], op0=Alu.mult,
                    op1=Alu.mult)
                out_sb = bl.tile([128, CS], F32, tag="outsb")
                nc.vector.tensor_mul(out_sb[:], ps2[:], w_bc[:])
                nc.vector.tensor_add(out_sb[:], out_sb[:], mfk[:])
                nc.sync.dma_start(
                    pair_rs["in"][:, slot * CS:(slot + 1) * CS], out_sb[:])

            def emit_rs(k, pair_rs):
                rs_out = dram.tile([NCH // G, 2 * CS], F32, tag="rso")
                nc.gpsimd.collective_compute(
                    "ReduceScatter", Alu.add, replica_groups=groups,
                    ins=[pair_rs["in"].opt()], outs=[rs_out.opt()])
                nc.sync.dma_start(
                    out_ext[:, (k - 1) * CS:(k + 1) * CS], rs_out[:])

            # ---------------- software pipeline ----------------
            g1 = {0: emit_gemm1(0)}
            with tc.tile_pool(name="psT", bufs=2, space="PSUM") as psT:
                for d in range(9):
                    for t in range(LT):
                        pt = psT.tile([128, 128], BF, tag="psT")
                        nc.tensor.transpose(
                            pt[:], kernT[:, d, t * 128:(t + 1) * 128],
                            ident_b[:])
                        nc.scalar.activation(
                            kern_lc[:, d, t, :], pt[:], Act.Identity,
                            scale=rnorm_col[:, t:t + 1])
            mp = {0: emit_maxpath(0, g1[0][1])}
            ag_pairs = {}
            rs_pairs = {}
            ps2s = {}
            blf = {}
            for k in range(NCHUNK + 3):
                if 0 <= k - 2 <= NCHUNK - 1:
                    blf[k - 2] = emit_blend_prefetch(k - 2)
                    ps2s[k - 2] = emit_gemm2(k - 2)
                if 0 <= k - 3 <= NCHUNK - 1:
                    j = k - 3
                    w_bc = emit_combine(j, ag_pairs[j // 2])
                    mbc, fgc = blf.pop(j)
                    emit_blend(j, ps2s.pop(j), w_bc, mbc, fgc,
                               rs_pairs[j // 2])
                    if j % 2 == 1:
                        emit_rs(j, rs_pairs.pop(j // 2))
                        ag_pairs.pop(j // 2)
                if k <= NCHUNK - 1:
                    if k % 2 == 0:
                        ag_pairs[k // 2] = {
                            "in": dram.tile([4 * CS], F32, tag="agi",
                                            name=f"agi{k // 2}")}
                        rs_pairs[k // 2] = {
                            "in": dram.tile([NCH, 2 * CS], F32, tag="rsi",
                                            name=f"rsi{k // 2}")}
                    emit_subexp(k, g1[k][0], mp[k])
                if k + 1 <= NCHUNK - 1:
                    g1[k + 1] = emit_gemm1(k + 1)
                    mp[k + 1] = emit_maxpath(k + 1, g1[k + 1][1])
                if k <= NCHUNK - 1:
                    emit_sums(k, ag_pairs[k // 2]["in"], mp.pop(k))
                    g1.pop(k)
                    if k % 2 == 1:
                        emit_ag(ag_pairs[k // 2])

            ctx_bl.__exit__(None, None, None)
            ctx_st.__exit__(None, None, None)
            ctx_scs.__exit__(None, None, None)

    nc.compile()
    return nc


def _shard_inputs(fg, mk):
    """fg [2,128,64,64] f32, mk [2,1,64,64] f32 -> per-core input maps."""
    in_maps = []
    for core in range(NCORES):
        b, r = core // G, core % G
        y0 = r * (W // G)
        feat = np.ascontiguousarray(fg[b].reshape(NCH, S), np.float32)
        mask = np.ascontiguousarray(mk[b].reshape(1, S), np.float32)
        band = np.zeros((NCH, 18, H), np.float32)
        mband = np.zeros((1, 18, H), np.float32)
        lo = y0 - 1
        src_lo = max(0, lo)
        src_hi = min(W, y0 + 17)
        band[:, src_lo - lo:src_hi - lo] = fg[b][:, src_lo:src_hi]
        mband[:, src_lo - lo:src_hi - lo] = mk[b][:, src_lo:src_hi]
        in_maps.append({
            "fg": feat,
            "fgband": np.ascontiguousarray(band.reshape(NCH, 18 * H)),
            "mask": mask,
            "maskband": np.ascontiguousarray(mband.reshape(1, 18 * H)),
        })
    return in_maps


def kernel(foreground, masks):
    global LAST_EXEC_TIME_NS
    from concourse.bass_utils import run_bass_kernel_spmd

    fg = np.asarray(foreground, np.float32)
    mk = np.asarray(masks, np.float32)
    assert fg.shape == (B, NCH, W, H) and mk.shape == (B, 1, W, H)

    nc = _CACHE.get("nc")
    if nc is None:
        nc = _build()
        _CACHE["nc"] = nc

    in_maps = _shard_inputs(fg, mk)
    trace = bool(os.environ.get("BASS_KERNEL_TRACE"))
    res = run_bass_kernel_spmd(nc, in_maps, core_ids=list(range(NCORES)),
                               trace=trace)
    LAST_EXEC_TIME_NS = res.exec_time_ns
    if res.exec_time_ns is not None:
        print(f"HW exec time: {res.exec_time_ns} ns")

    out = np.empty((B, NCH, W, H), np.float32)
    for core in range(NCORES):
        b, r = core // G, core % G
        out[b, 32 * r:32 * (r + 1)] = (
            res.results[core]["out"].reshape(32, W, H))
    return out


# revision 10
# speedup vs baseline: 1.1773x; 1.0200x over previous
"""Distributed Trainium2 Bass kernel for the contextual-attention module.

Sharding (per hint): data-parallel over batch (2 samples x 4 cores); within a
sample the L=4096 kernel axis is sharded 4 ways (1024 kernels / 16 patch-center
rows per core).  Per core, the 64x64 spatial grid is processed in 8 chunks of
8 rows (CS=512 positions):

  GEMM1  scores[l, s] = sum_{c,d} kern[l,c,d] * boxfeat[c, s+d]   (the 3x3
         box-sum of scores is commuted onto the feature map).  fp8 DoubleRow:
         the 9 shift-terms run as 4 paired matmuls (shift pairs pre-staged as
         planes of bp_all) + 1 single.  Kernel L2 normalization rides as a
         per-partition activation scale on the PSUM->SBUF copy (rnorm[l]).
  softmax over the full L axis is flash-style: local max via one gpsimd
         partition_all_reduce, exp against it (written into a single fp8
         attn map ahalo_all [128, 8t, 66, 66]), local sums via fp8 DoubleRow
         ones-matmuls pairing l-tiles; (max, sum) stat rows for chunk pairs
         go through one 4-core AllGather + local combine with ~2 periods of
         latency slack.
  GEMM2  is output-pixel-major: for each output chunk, 9 shifted windows x
         4 l-tile-pairs (36 fp8 DoubleRow matmuls, kernel index flipped,
         rnorm folded into kern_lc) accumulate into ONE PSUM tile -- no
         canvas overlap-add.
  blend  out = psum * (fac * (1-mask)/9) + feat*mask/4, per-s factor
         broadcast once; per chunk-pair ReduceScatter over channels.

The PE instruction stream (GEMM2(k-2), GEMM1(k+1), sums(k) per iteration)
never waits on a collective, keeping the HAM clock gate warm.
"""

import os
import sys
import types

for _p in ("/opt/trn_rl_repo",):
    if os.path.isdir(_p) and _p not in sys.path:
        sys.path.append(_p)


def _ensure_axon_hooks():
    """Make antenv.axon_hooks importable so bass_utils trace mode never
    crashes on the import (hook may still be None -> tracing is skipped)."""
    try:
        import antenv.axon_hooks  # noqa: F401
        return
    except Exception:
        pass
    try:
        import antenv
        mod = types.ModuleType("antenv.axon_hooks")
        mod._hook = None

        def set_axon_ntff_profile_hook(hook):
            mod._hook = hook

        def get_axon_ntff_profile_hook():
            return mod._hook

        mod.set_axon_ntff_profile_hook = set_axon_ntff_profile_hook
        mod.get_axon_ntff_profile_hook = get_axon_ntff_profile_hook
        sys.modules["antenv.axon_hooks"] = mod
        antenv.axon_hooks = mod
    except Exception:
        pass


_ensure_axon_hooks()

import numpy as np  # noqa: E402

NCH = 128           # channels
W = H = 64          # spatial
S = W * H           # 4096 spatial positions
B = 2               # batch
G = 4               # cores per sample
NCORES = 8
LS = S // G         # kernels per core (1024)
LT = LS // 128      # l-tiles per core (8)
ROWS = 8            # patch-center rows per chunk
CS = ROWS * H       # spatial chunk (512)
NCHUNK = W // ROWS  # 8 chunks
EPS = 1e-7

# fp8 DoubleRow toggles (fall back to bf16 if accuracy demands)
FP8_G1 = False       # scores GEMM in fp8 (pairs shift terms)
FP8_G2 = False       # transpose-conv GEMM + sums in fp8 (pairs l-tiles)

_CACHE = {}
LAST_EXEC_TIME_NS = None

# d-pair table for GEMM1: pairs of shift indices (d = 3*dy + dx) and the
# flat 66x66 offset delta between the two windows of each pair
D_PAIRS = [(0, 1), (2, 3), (4, 5), (6, 7)]
D_DELTA = [1, 64, 1, 1]


def _build():
    from concourse import bacc, bass_isa, tile, mybir
    from concourse.masks import make_identity

    F32 = mybir.dt.float32
    BF = mybir.dt.bfloat16
    F8 = mybir.dt.float8e4
    DR = mybir.MatmulPerfMode.DoubleRow
    Alu = mybir.AluOpType
    Act = mybir.ActivationFunctionType
    AxX = mybir.AxisListType.X
    RMax = bass_isa.ReduceOp.max

    G2DT = F8 if FP8_G2 else BF

    nc = bacc.Bacc("TRN2", target_bir_lowering=False, debug=False,
                   num_devices=NCORES)

    fg_ext = nc.dram_tensor("fg", [NCH, S], F32, kind="ExternalInput")
    fgband_ext = nc.dram_tensor("fgband", [NCH, 18 * H], F32,
                                kind="ExternalInput")
    mask_ext = nc.dram_tensor("mask", [1, S], F32, kind="ExternalInput")
    mband_ext = nc.dram_tensor("maskband", [1, 18 * H], F32,
                               kind="ExternalInput")
    out_ext = nc.dram_tensor("out", [NCH // G, S], F32, kind="ExternalOutput")

    groups = [[0, 1, 2, 3], [4, 5, 6, 7]]

    with tile.TileContext(nc) as tc:
        with tc.tile_pool(name="const", bufs=1) as cpool, \
             tc.tile_pool(name="pers", bufs=1) as pers, \
             tc.tile_pool(name="psA", bufs=2, space="PSUM") as psA, \
             tc.tile_pool(name="psB", bufs=2, space="PSUM") as psB, \
             tc.tile_pool(name="psS", bufs=2, space="PSUM") as psS, \
             tc.tile_pool(name="dram", bufs=2, space="DRAM") as dram, \
             tc.tile_pool(name="dramP", bufs=1, space="DRAM") as dramP:

            ident_b = cpool.tile([128, 128], BF, tag="idb")
            make_identity(nc, ident_b[:])
            ones_s = cpool.tile([128, 1], G2DT, tag="ones")
            nc.gpsimd.memset(ones_s[:], 1.0)

            # ---------------- persistent tensors ----------------
            if FP8_G1:
                # paired box-feature planes: plane 2i = boxfeat, plane 2i+1 =
                # boxfeat shifted by D_DELTA[i] (flat), so one DoubleRow AP
                # covers both windows of d-pair i
                bp_all = pers.tile([NCH, 8, 66, 66], F8, tag="bp")
                kern8 = pers.tile([NCH, 9, LS], F8, tag="kern8")
            else:
                bp_all = pers.tile([NCH, 66, 66], BF, tag="bp")
                kern8 = pers.tile([NCH, 9, LS], BF, tag="kern8")
            kern_lc = pers.tile([128, 9, LT, NCH], G2DT, tag="kernlc")
            ahalo = pers.tile([128, LT, 66, 66], G2DT, tag="ahalo")
            q32 = pers.tile([32, NCHUNK, CS // 32], F32, tag="q32")
            rnorm_col = pers.tile([128, LT], F32, tag="rnorm")

            bar_in = dramP.tile([16], F32, tag="bari")
            bar_out = dramP.tile([16 * NCORES], F32, tag="baro")
            bar2_in = dramP.tile([4 * CS], F32, tag="bari2")
            bar2_out = dramP.tile([4 * CS * G], F32, tag="baro2")
            nrm_dram = dramP.tile([LS], F32, tag="nrmd")

            with tc.tile_pool(name="prep", bufs=1) as prep:
                # ---- input loads ----
                mband_row = prep.tile([1, 18 * H], F32, tag="mbandrow")
                nc.sync.dma_start(mband_row[:], mband_ext[:])
                fgband_sb = prep.tile([NCH, 18, H], F32, tag="fgband")
                nc.sync.dma_start(
                    fgband_sb[:],
                    fgband_ext[:].rearrange("c (r x) -> c r x", r=18))
                fg_sb = prep.tile([NCH, W, H], F32, tag="fgsb")
                nc.sync.dma_start(
                    fg_sb[:], fg_ext[:].rearrange("c (y x) -> c y x", y=W))
                m32a = prep.tile([32, NCHUNK, CS // 32], F32, tag="m32a")
                for k in range(NCHUNK):
                    nc.sync.dma_start(m32a[:, k, :],
                                      mask_ext[:, k * CS:(k + 1) * CS])

                # ---- warmup collectives (absorb cold-start early) ----
                nc.gpsimd.dma_start(bar_in[:], mband_row[0:1, 0:16])
                nc.gpsimd.collective_compute(
                    "AllGather", Alu.bypass,
                    replica_groups=[list(range(NCORES))],
                    ins=[bar_in.opt()], outs=[bar_out.opt()])
                # warm the 4-core communicator with the REAL stats-AG size
                junk = prep.tile([32, 64], F32, tag="junk")
                nc.gpsimd.memset(junk[:], 0.0)
                nc.gpsimd.dma_start(bar2_in[:], junk[:])
                nc.gpsimd.collective_compute(
                    "AllGather", Alu.bypass, replica_groups=groups,
                    ins=[bar2_in.opt()], outs=[bar2_out.opt()])

                # ---- kernels: kernT[c, d, l] = (band*mask)[shifted] + EPS --
                mband_bc = prep.tile([NCH, 18 * H], BF, tag="mbandbc")
                mband_bf = prep.tile([1, 18 * H], BF, tag="mbandbf")
                nc.scalar.activation(mband_bf[:], mband_row[:], Act.Identity)
                nc.gpsimd.partition_broadcast(mband_bc[:], mband_bf[:])
                bgbandp = prep.tile([NCH, 18, 66], F32, tag="bgbandp")
                nc.gpsimd.memset(bgbandp[:], 0.0)
                nc.vector.tensor_mul(
                    bgbandp[:, :, 1:65], fgband_sb[:],
                    mband_bc[:].rearrange("c (r x) -> c r x", r=18))
                kernT = prep.tile([NCH, 9, LS], BF, tag="kernT")
                for d in range(9):
                    dy, dx = d // 3, d % 3
                    nc.vector.tensor_scalar_add(
                        kernT[:, d, :],
                        bgbandp[:, dy:dy + 16, dx:dx + 64], EPS)
                # fp8 copy for the score GEMM (bf16 kernT feeds norms +
                # transposes)
                for d in range(9):
                    nc.scalar.activation(kern8[:, d, :], kernT[:, d, :],
                                         Act.Identity)

                # ---- kernel norms: sumsq via squares + ones-matmul ----
                onesb = prep.tile([128, 1], BF, tag="onesb")
                nc.gpsimd.memset(onesb[:], 1.0)
                ksq = prep.tile([NCH, LS], BF, tag="ksq")
                ps_s0 = psS.tile([1, 512], F32, tag="psS")
                ps_s1 = psS.tile([1, 512], F32, tag="psS")
                for d in range(9):
                    nc.vector.tensor_mul(ksq[:], kernT[:, d, :],
                                         kernT[:, d, :])
                    nc.tensor.matmul(ps_s0[:], onesb[:], ksq[:, 0:512],
                                     start=(d == 0), stop=(d == 8))
                    nc.tensor.matmul(ps_s1[:], onesb[:], ksq[:, 512:1024],
                                     start=(d == 0), stop=(d == 8))
                srow = prep.tile([1, LS], F32, tag="srow")
                nc.scalar.activation(srow[:, 0:512], ps_s0[:], Act.Identity)
                nc.scalar.activation(srow[:, 512:1024], ps_s1[:],
                                     Act.Identity)
                # flatten [1, 1024] -> [128, 8] (partition p, col t <-
                # l = t*128 + p) via a DRAM bounce
                nc.gpsimd.dma_start(nrm_dram[:], srow[:])
                sq128 = prep.tile([128, LT], F32, tag="sq128")
                nc.gpsimd.dma_start(
                    sq128[:],
                    nrm_dram[:].rearrange("(t p) -> p t", p=128))
                norm128 = prep.tile([128, LT], F32, tag="norm128")
                nc.scalar.activation(norm128[:], sq128[:], Act.Sqrt)
                nc.vector.reciprocal(rnorm_col[:], norm128[:])

                # ---- blend constant: q32 = (1-mask)/9 in stat layout ----
                nc.vector.tensor_scalar(q32[:], m32a[:], -1.0 / 9.0,
                                        1.0 / 9.0, op0=Alu.mult, op1=Alu.add)

                # ---- box filter via zero-padded shifts (no edge copies) --
                fgbp = prep.tile([NCH, W, 68], BF, tag="fgbp")
                nc.gpsimd.memset(fgbp[:], 0.0)
                nc.scalar.activation(fgbp[:, :, 2:66], fg_sb[:], Act.Identity)
                t1 = prep.tile([NCH, W, 66], BF, tag="t1")
                nc.vector.tensor_add(t1[:], fgbp[:, :, 0:66],
                                     fgbp[:, :, 1:67])
                tmpHp = prep.tile([NCH, 68, 66], BF, tag="tmpHp")
                nc.gpsimd.memset(tmpHp[:], 0.0)
                nc.vector.tensor_add(tmpHp[:, 2:66, :], t1[:],
                                     fgbp[:, :, 2:68])
                t2 = prep.tile([NCH, 66, 66], BF, tag="t1")
                nc.vector.tensor_add(t2[:], tmpHp[:, 0:66, :],
                                     tmpHp[:, 1:67, :])
                if FP8_G1:
                    boxbf = prep.tile([NCH, 66, 66], BF, tag="boxbf")
                    nc.vector.tensor_add(boxbf[:], t2[:], tmpHp[:, 2:68, :])
                    box8 = prep.tile([NCH, 66, 66], F8, tag="box8")
                    nc.scalar.activation(box8[:], boxbf[:], Act.Identity)
                    # stage the 8 planes: 2i = box8, 2i+1 = box8 shifted
                    b8f = box8[:].rearrange("c y x -> c (y x)")
                    bpf = bp_all[:].rearrange("c p y x -> c p (y x)")
                    for i, delta in enumerate(D_DELTA):
                        nc.sync.dma_start(bpf[:, 2 * i, :], b8f)
                        nc.scalar.dma_start(
                            bpf[:, 2 * i + 1, 0:4356 - delta],
                            b8f[:, delta:4356])
                else:
                    nc.vector.tensor_add(bp_all[:], t2[:], tmpHp[:, 2:68, :])

                # ---- zero the attn-map borders (rest is overwritten) ----
                nc.gpsimd.memset(ahalo[:, :, 0, :], 0.0)
                nc.gpsimd.memset(ahalo[:, :, 65, :], 0.0)
                nc.gpsimd.memset(ahalo[:, :, :, 0], 0.0)
                nc.gpsimd.memset(ahalo[:, :, :, 65], 0.0)

                # ---- kern_lc[l, d, c] via PE transposes, rnorm folded ----
                with tc.tile_pool(name="psT", bufs=2, space="PSUM") as psT:
                    for d in range(9):
                        for t in range(LT):
                            pt = psT.tile([128, 128], BF, tag="psT")
                            nc.tensor.transpose(
                                pt[:], kernT[:, d, t * 128:(t + 1) * 128],
                                ident_b[:])
                            nc.scalar.activation(
                                kern_lc[:, d, t, :], pt[:], Act.Identity,
                                scale=rnorm_col[:, t:t + 1])

            # ---------------- chunk-loop pools ----------------
            ctx_scs = tc.tile_pool(name="scs", bufs=2)
            scsp = ctx_scs.__enter__()
            ctx_st = tc.tile_pool(name="stat", bufs=2)
            st = ctx_st.__enter__()
            ctx_bl = tc.tile_pool(name="blend", bufs=2)
            bl = ctx_bl.__enter__()

            def emit_gemm1(k):
                """scores for chunk k -> scs (normalized) + mtmp (max)."""
                r0 = k * ROWS
                scs = scsp.tile([128, LT, CS], F32, tag="scs")
                mtmp = st.tile([128, CS], F32, tag="mtmp")
                for t in range(LT):
                    ps = psA.tile([128, CS], F32, tag="psA")
                    if FP8_G1:
                        for i, (d0, d1) in enumerate(D_PAIRS):
                            dy, dx = d0 // 3, d0 % 3
                            nc.tensor.matmul(
                                ps[:],
                                kern8[:, d0:d0 + 2, t * 128:(t + 1) * 128],
                                bp_all[:, 2 * i:2 * i + 2,
                                       r0 + dy:r0 + dy + ROWS, dx:dx + 64],
                                start=(i == 0), stop=False, perf_mode=DR)
                        nc.tensor.matmul(
                            ps[:],
                            kern8[:, 8, t * 128:(t + 1) * 128],
                            bp_all[:, 0, r0 + 2:r0 + 2 + ROWS, 2:66],
                            start=False, stop=True)
                    else:
                        for d in range(9):
                            dy, dx = d // 3, d % 3
                            nc.tensor.matmul(
                                ps[:],
                                kern8[:, d, t * 128:(t + 1) * 128],
                                bp_all[:, r0 + dy:r0 + dy + ROWS, dx:dx + 64],
                                start=(d == 0), stop=(d == 8))
                    nc.scalar.activation(scs[:, t, :], ps[:], Act.Identity,
                                         scale=rnorm_col[:, t:t + 1])
                    if t == 0:
                        nc.vector.tensor_copy(mtmp[:], scs[:, 0, :])
                    else:
                        nc.vector.scalar_tensor_tensor(
                            mtmp[:], scs[:, t, :], 1.0, mtmp[:],
                            op0=Alu.mult, op1=Alu.max)
                return scs, mtmp

            def emit_maxpath(k, mtmp):
                m_bc = st.tile([128, CS], F32, tag="mbc")
                nc.gpsimd.partition_all_reduce(m_bc[:], mtmp[:], 128, RMax)
                return m_bc

            def emit_subexp(k, scs, m_bc):
                r0 = k * ROWS
                for t in range(LT):
                    diff = st.tile([128, CS], F32, tag="diff")
                    nc.vector.tensor_sub(diff[:], scs[:, t, :], m_bc[:])
                    nc.scalar.activation(
                        ahalo[:, t, 1 + r0:9 + r0, 1:65],
                        diff[:].rearrange("p (r x) -> p r x", r=ROWS),
                        Act.Exp)

            def emit_sums(k, ag_in, m_bc):
                r0 = k * ROWS
                slot = k % 2
                ps_sum = psS.tile([1, CS], F32, tag="psS")
                for t in range(LT):
                    nc.tensor.matmul(
                        ps_sum[:], ones_s[:],
                        ahalo[:, t, 1 + r0:9 + r0, 1:65],
                        start=(t == 0), stop=(t == LT - 1))
                s_row = st.tile([1, CS], F32, tag="srowc")
                nc.scalar.activation(s_row[:], ps_sum[:], Act.Identity)
                nc.gpsimd.dma_start(
                    ag_in[slot * 2 * CS:slot * 2 * CS + CS], m_bc[0:1, :])
                nc.gpsimd.dma_start(
                    ag_in[slot * 2 * CS + CS:(slot + 1) * 2 * CS], s_row[:])

            def emit_ag(pair):
                ag_out = dram.tile([4 * CS * G], F32, tag="ago")
                nc.gpsimd.collective_compute(
                    "AllGather", Alu.bypass, replica_groups=groups,
                    ins=[pair["in"].opt()], outs=[ag_out.opt()])
                pair["out"] = ag_out

            def emit_combine(k, pair):
                """gathered stats -> w_bc = fac * (1-mask)/9 broadcast.
                [32, 16] stat layout (linear col order, like the dumps)."""
                slot = k % 2
                ag_in, ag_out = pair["in"], pair["out"]
                cm = st.tile([32, G, CS // 32], F32, tag="cm")
                cs = st.tile([32, G, CS // 32], F32, tag="cs")
                for r in range(G):
                    base = r * 4 * CS + slot * 2 * CS
                    nc.gpsimd.dma_start(cm[:, r, :], ag_out[base:base + CS])
                    nc.gpsimd.dma_start(cs[:, r, :],
                                        ag_out[base + CS:base + 2 * CS])
                m32 = st.tile([32, CS // 32], F32, tag="m32")
                nc.gpsimd.dma_start(
                    m32[:], ag_in[slot * 2 * CS:slot * 2 * CS + CS])
                Mx = st.tile([32, CS // 32], F32, tag="Mx")
                nc.vector.tensor_reduce(
                    Mx[:], cm[:].rearrange("p r t -> p t r"), AxX, Alu.max)
                for r in range(G):
                    nc.vector.tensor_sub(cm[:, r, :], cm[:, r, :], Mx[:])
                nc.scalar.activation(cm[:], cm[:], Act.Exp)
                nc.vector.tensor_mul(cs[:], cs[:], cm[:])
                gs = st.tile([32, CS // 32], F32, tag="gs")
                nc.vector.tensor_reduce(
                    gs[:], cs[:].rearrange("p r t -> p t r"), AxX, Alu.add)
                rg = st.tile([32, CS // 32], F32, tag="rg")
                nc.vector.reciprocal(rg[:], gs[:])
                w_sl = st.tile([32, CS // 32], F32, tag="wsl")
                nc.vector.tensor_sub(w_sl[:], m32[:], Mx[:])
                nc.scalar.activation(w_sl[:], w_sl[:], Act.Exp)
                nc.vector.tensor_mul(w_sl[:], w_sl[:], rg[:])
                nc.vector.tensor_mul(w_sl[:], w_sl[:], q32[:, k, :])
                w_dram = dram.tile([CS], F32, tag="wd")
                nc.gpsimd.dma_start(w_dram[:], w_sl[:])
                w_row = st.tile([1, CS], F32, tag="wrow")
                nc.gpsimd.dma_start(w_row[:], w_dram[:])
                w_bc = st.tile([128, CS], F32, tag="wbc")
                nc.gpsimd.partition_broadcast(w_bc[:], w_row[:])
                return w_bc

            def emit_blend_prefetch(k):
                mrowk = bl.tile([1, CS], F32, tag="mrowk")
                nc.sync.dma_start(mrowk[:], mask_ext[:, k * CS:(k + 1) * CS])
                mbc = bl.tile([128, CS], F32, tag="mbck")
                nc.gpsimd.partition_broadcast(mbc[:], mrowk[:])
                fgc = bl.tile([NCH, CS], F32, tag="fgc")
                nc.sync.dma_start(fgc[:], fg_ext[:, k * CS:(k + 1) * CS])
                return mbc, fgc

            def emit_gemm2(k):
                r0 = k * ROWS
                ps2 = psB.tile([128, CS], F32, tag="psB")
                n = 0
                nmax = 36 if FP8_G2 else 72
                for dyp in range(3):
                    for dxp in range(3):
                        dflip = (2 - dyp) * 3 + (2 - dxp)
                        if FP8_G2:
                            for tp in range(LT // 2):
                                nc.tensor.matmul(
                                    ps2[:],
                                    kern_lc[:, dflip, 2 * tp:2 * tp + 2, :],
                                    ahalo[:, 2 * tp:2 * tp + 2,
                                          r0 + dyp:r0 + dyp + ROWS,
                                          dxp:dxp + 64],
                                    start=(n == 0), stop=(n == nmax - 1),
                                    perf_mode=DR)
                                n += 1
                        else:
                            for t in range(LT):
                                nc.tensor.matmul(
                                    ps2[:],
                                    kern_lc[:, dflip, t, :],
                                    ahalo[:, t, r0 + dyp:r0 + dyp + ROWS,
                                          dxp:dxp + 64],
                                    start=(n == 0), stop=(n == nmax - 1))
                                n += 1
                return ps2

            def emit_blend(k, ps2, w_bc, mbc, fgc, pair_rs):
                slot = k % 2
                mfk = bl.tile([128, CS], F32, tag="mfk")
                nc.vector.scalar_tensor_tensor(
                    mfk[:], fgc[:], 1.0 / G, mbc[:], op0=Alu.mult,
                    op1=Alu.mult)
                out_sb = bl.tile([128, CS], F32, tag="outsb")
                nc.vector.tensor_mul(out_sb[:], ps2[:], w_bc[:])
                nc.vector.tensor_add(out_sb[:], out_sb[:], mfk[:])
                nc.sync.dma_start(
                    pair_rs["in"][:, slot * CS:(slot + 1) * CS], out_sb[:])

            def emit_rs(k, pair_rs):
                rs_out = dram.tile([NCH // G, 2 * CS], F32, tag="rso")
                nc.gpsimd.collective_compute(
                    "ReduceScatter", Alu.add, replica_groups=groups,
                    ins=[pair_rs["in"].opt()], outs=[rs_out.opt()])
                nc.sync.dma_start(
                    out_ext[:, (k - 1) * CS:(k + 1) * CS], rs_out[:])

            # ---------------- software pipeline ----------------
            g1 = {0: emit_gemm1(0)}
            mp = {0: emit_maxpath(0, g1[0][1])}
            ag_pairs = {}
            rs_pairs = {}
            ps2s = {}
            blf = {}
            for k in range(NCHUNK + 2):
                if 0 <= k - 2 <= NCHUNK - 1:
                    blf[k - 2] = emit_blend_prefetch(k - 2)
                    ps2s[k - 2] = emit_gemm2(k - 2)
                # last iteration blends both remaining chunks
                jbs = [k - 3, k - 2] if k == NCHUNK + 1 else [k - 3]
                for jb in jbs:
                    if not (0 <= jb <= NCHUNK - 1):
                        continue
                    w_bc = emit_combine(jb, ag_pairs[jb // 2])
                    mbc, fgc = blf.pop(jb)
                    emit_blend(jb, ps2s.pop(jb), w_bc, mbc, fgc,
                               rs_pairs[jb // 2])
                    if jb % 2 == 1:
                        emit_rs(jb, rs_pairs.pop(jb // 2))
                        ag_pairs.pop(jb // 2)
                if k <= NCHUNK - 1:
                    if k % 2 == 0:
                        ag_pairs[k // 2] = {
                            "in": dram.tile([4 * CS], F32, tag="agi",
                                            name=f"agi{k // 2}")}
                        rs_pairs[k // 2] = {
                            "in": dram.tile([NCH, 2 * CS], F32, tag="rsi",
                                            name=f"rsi{k // 2}")}
                    emit_subexp(k, g1[k][0], mp[k])
                if k + 1 <= NCHUNK - 1:
                    g1[k + 1] = emit_gemm1(k + 1)
                    mp[k + 1] = emit_maxpath(k + 1, g1[k + 1][1])
                if k <= NCHUNK - 1:
                    emit_sums(k, ag_pairs[k // 2]["in"], mp.pop(k))
                    g1.pop(k)
                    if k % 2 == 1:
                        emit_ag(ag_pairs[k // 2])

            ctx_bl.__exit__(None, None, None)
            ctx_st.__exit__(None, None, None)
            ctx_scs.__exit__(None, None, None)

    nc.compile()
    return nc


def _shard_inputs(fg, mk):
    """fg [2,128,64,64] f32, mk [2,1,64,64] f32 -> per-core input maps."""
    in_maps = []
    for core in range(NCORES):
        b, r = core // G, core % G
        y0 = r * (W // G)
        feat = np.ascontiguousarray(fg[b].reshape(NCH, S), np.float32)
        mask = np.ascontiguousarray(mk[b].reshape(1, S), np.float32)
        band = np.zeros((NCH, 18, H), np.float32)
        mband = np.zeros((1, 18, H), np.float32)
        lo = y0 - 1
        src_lo = max(0, lo)
        src_hi = min(W, y0 + 17)
        band[:, src_lo - lo:src_hi - lo] = fg[b][:, src_lo:src_hi]
        mband[:, src_lo - lo:src_hi - lo] = mk[b][:, src_lo:src_hi]
        in_maps.append({
            "fg": feat,
            "fgband": np.ascontiguousarray(band.reshape(NCH, 18 * H)),
            "mask": mask,
            "maskband": np.ascontiguousarray(mband.reshape(1, 18 * H)),
        })
    return in_maps


def kernel(foreground, masks):
    global LAST_EXEC_TIME_NS
    from concourse.bass_utils import run_bass_kernel_spmd

    fg = np.asarray(foreground, np.float32)
    mk = np.asarray(masks, np.float32)
    assert fg.shape == (B, NCH, W, H) and mk.shape == (B, 1, W, H)

    nc = _CACHE.get("nc")
    if nc is None:
        nc = _build()
        _CACHE["nc"] = nc

    in_maps = _shard_inputs(fg, mk)
    trace = bool(os.environ.get("BASS_KERNEL_TRACE"))
    res = run_bass_kernel_spmd(nc, in_maps, core_ids=list(range(NCORES)),
                               trace=trace)
    LAST_EXEC_TIME_NS = res.exec_time_ns
    if res.exec_time_ns is not None:
        print(f"HW exec time: {res.exec_time_ns} ns")

    out = np.empty((B, NCH, W, H), np.float32)
    for core in range(NCORES):
        b, r = core // G, core % G
        out[b, 32 * r:32 * (r + 1)] = (
            res.results[core]["out"].reshape(32, W, H))
    return out


# revision 18
# speedup vs baseline: 1.3913x; 1.1818x over previous
"""Distributed Trainium2 Bass kernel for the contextual-attention module.

Sharding (per hint): data-parallel over batch (2 samples x 4 cores); within a
sample the L=4096 kernel axis is sharded 4 ways (1024 kernels / 16 patch-center
rows per core).  Per core, the 64x64 spatial grid is processed in 8 chunks of
8 rows (CS=512 positions).

fp8 DoubleRow geometry: the box-filtered feature map and the attn maps are
stored "spaced" -- one fp8 value every 2 bytes, rows 132 B apart -- so every
3x3-shift window becomes a flat stride-2 byte stream with an even start
offset, which satisfies the dual-fp8 ISA rules (rhs free-AP depth <= 2, 2B
start alignment).  An out-of-range x shift wraps into the neighbouring
zeroed slot, so no halo copies are needed.

  GEMM1  scores[l, s] = sum_{c,d} kern[l,c,d] * boxfeat[c, s+d]  (3x3
         box-sum commuted onto the feature map); per l-tile and half-chunk,
         4 DoubleRow matmuls (paired shifts) + 1 single accumulate in PSUM.
         Kernel L2 normalization rides as a per-partition activation scale
         on the PSUM->SBUF copy (rnorm[l]).
  softmax over the full L axis is flash-style: local max via one gpsimd
         partition_all_reduce, exp against it (strided write into the fp8
         attn map), local sums via fp8 ones-matmuls; (max, sum) stat rows
         for chunk pairs go through one 4-core AllGather + local combine
         with ~2 pipeline periods of latency slack.
  GEMM2  is output-pixel-major: per half-chunk, 9 shifted windows x 4
         l-tile-pairs of DoubleRow matmuls (kernel index flipped, rnorm
         folded into kern_lc) accumulate into one PSUM tile -- no canvas
         overlap-add.
  blend  out = psum * (fac * (1-mask)/9) + feat*mask/4; ReduceScatter over
         channels per chunk pair (last two chunks scatter individually to
         shorten the tail).

The PE instruction stream (GEMM2(k-2), GEMM1(k+1), sums(k) per iteration)
never waits on a collective, keeping the HAM clock gate warm.
"""

import os
import sys
import types

for _p in ("/opt/trn_rl_repo",):
    if os.path.isdir(_p) and _p not in sys.path:
        sys.path.append(_p)


def _ensure_axon_hooks():
    """Make antenv.axon_hooks importable so bass_utils trace mode never
    crashes on the import (hook may still be None -> tracing is skipped)."""
    try:
        import antenv.axon_hooks  # noqa: F401
        return
    except Exception:
        pass
    try:
        import antenv
        mod = types.ModuleType("antenv.axon_hooks")
        mod._hook = None

        def set_axon_ntff_profile_hook(hook):
            mod._hook = hook

        def get_axon_ntff_profile_hook():
            return mod._hook

        mod.set_axon_ntff_profile_hook = set_axon_ntff_profile_hook
        mod.get_axon_ntff_profile_hook = get_axon_ntff_profile_hook
        sys.modules["antenv.axon_hooks"] = mod
        antenv.axon_hooks = mod
    except Exception:
        pass


_ensure_axon_hooks()

import numpy as np  # noqa: E402

NCH = 128           # channels
W = H = 64          # spatial
S = W * H           # 4096 spatial positions
B = 2               # batch
G = 4               # cores per sample
NCORES = 8
LS = S // G         # kernels per core (1024)
LT = LS // 128      # l-tiles per core (8)
ROWS = 8            # patch-center rows per chunk
CS = ROWS * H       # spatial chunk (512)
NCHUNK = W // ROWS  # 8 chunks
EPS = 1e-7

ROWB = 132          # spaced-row pitch in bytes (66 fp8 slots * 2)
PL = 2 + 66 * ROWB + 2   # spaced plane size (lead pad + 66 rows + tail)
NSTR = 263          # stream length per half-window (3*66 + 65)

_CACHE = {}
LAST_EXEC_TIME_NS = None

# d-pair table for the score GEMM: shift pairs (d = 3*dy + dx) and the byte
# delta between the two windows of each pair in the spaced layout
D_PAIRS = [(0, 1), (2, 3), (4, 5), (6, 7)]
D_DELTA = [2, 128, 2, 2]


def _build():
    from concourse import bacc, bass_isa, tile, mybir
    from concourse.ap import AP as RawAP
    from concourse.masks import make_identity

    F32 = mybir.dt.float32
    BF = mybir.dt.bfloat16
    F8 = mybir.dt.float8e4
    DRow = mybir.MatmulPerfMode.DoubleRow
    Alu = mybir.AluOpType
    Act = mybir.ActivationFunctionType
    AxX = mybir.AxisListType.X
    RMax = bass_isa.ReduceOp.max
    RAdd = bass_isa.ReduceOp.add

    nc = bacc.Bacc("TRN2", target_bir_lowering=False, debug=False,
                   num_devices=NCORES)

    fg_ext = nc.dram_tensor("fg", [NCH, S], F32, kind="ExternalInput")
    fgband_ext = nc.dram_tensor("fgband", [NCH, 18 * H], F32,
                                kind="ExternalInput")
    mask_ext = nc.dram_tensor("mask", [1, S], F32, kind="ExternalInput")
    mband_ext = nc.dram_tensor("maskband", [1, 18 * H], F32,
                               kind="ExternalInput")
    out_ext = nc.dram_tensor("out", [NCH // G, S], F32, kind="ExternalOutput")

    groups = [[0, 1, 2, 3], [4, 5, 6, 7]]

    def rap(base, off, dims):
        """Raw strided view of a [128, N] tile at element offset `off`."""
        return RawAP(base.tensor, base.offset + off,
                     [list(base.ap[0])] + [list(d) for d in dims])

    with tile.TileContext(nc) as tc:
        with tc.tile_pool(name="const", bufs=1) as cpool, \
             tc.tile_pool(name="pers", bufs=1) as pers, \
             tc.tile_pool(name="psA", bufs=2, space="PSUM") as psA, \
             tc.tile_pool(name="psB", bufs=4, space="PSUM") as psB, \
             tc.tile_pool(name="psS", bufs=2, space="PSUM") as psS, \
             tc.tile_pool(name="dram", bufs=2, space="DRAM") as dram, \
             tc.tile_pool(name="dramP", bufs=1, space="DRAM") as dramP:

            ident_b = cpool.tile([128, 128], BF, tag="idb")
            make_identity(nc, ident_b[:])
            ones_s = cpool.tile([128, 1], F8, tag="ones")
            nc.gpsimd.memset(ones_s[:], 1.0)

            # ---------------- persistent tensors ----------------
            box_sp = pers.tile([NCH, PL], F8, tag="boxsp")
            kern8 = pers.tile([NCH, 9, LS], F8, tag="kern8")
            kern_lc = pers.tile([128, 9, LT, NCH], F8, tag="kernlc")
            ahalo = pers.tile([128, LT * PL], F8, tag="ahalo")
            q32 = pers.tile([32, NCHUNK, CS // 32], F32, tag="q32")
            rnorm_col = pers.tile([128, LT], F32, tag="rnorm")

            box_b = box_sp[:]
            ah_b = ahalo[:]

            bar_in = dramP.tile([16], F32, tag="bari")
            bar_out = dramP.tile([16 * NCORES], F32, tag="baro")
            bar2_in = dramP.tile([4 * CS], F32, tag="bari2")
            bar2_out = dramP.tile([4 * CS * G], F32, tag="baro2")
            nrm_dram = dramP.tile([LS], F32, tag="nrmd")

            with tc.tile_pool(name="prep", bufs=1) as prep:
                # ---- input loads ----
                mband_row = prep.tile([1, 18 * H], F32, tag="mbandrow")
                nc.sync.dma_start(mband_row[:], mband_ext[:])
                fgband_sb = prep.tile([NCH, 18, H], F32, tag="fgband")
                nc.sync.dma_start(
                    fgband_sb[:],
                    fgband_ext[:].rearrange("c (r x) -> c r x", r=18))
                fg_sb = prep.tile([NCH, W, H], F32, tag="fgsb")
                nc.sync.dma_start(
                    fg_sb[:], fg_ext[:].rearrange("c (y x) -> c y x", y=W))
                m32a = prep.tile([32, NCHUNK, CS // 32], F32, tag="m32a")
                for k in range(NCHUNK):
                    nc.sync.dma_start(m32a[:, k, :],
                                      mask_ext[:, k * CS:(k + 1) * CS])

                # ---- warmup collectives (absorb cold-start early) ----
                nc.gpsimd.dma_start(bar_in[:], mband_row[0:1, 0:16])
                nc.gpsimd.collective_compute(
                    "AllGather", Alu.bypass,
                    replica_groups=[list(range(NCORES))],
                    ins=[bar_in.opt()], outs=[bar_out.opt()])
                junk = prep.tile([32, 64], F32, tag="junk")
                nc.gpsimd.memset(junk[:], 0.0)
                nc.gpsimd.dma_start(bar2_in[:], junk[:])
                nc.gpsimd.collective_compute(
                    "AllGather", Alu.bypass, replica_groups=groups,
                    ins=[bar2_in.opt()], outs=[bar2_out.opt()])

                # ---- kernels: kernT[c, d, l] = (band*mask)[shifted] + EPS --
                mband_bc = prep.tile([NCH, 18 * H], BF, tag="mbandbc")
                mband_bf = prep.tile([1, 18 * H], BF, tag="mbandbf")
                nc.scalar.activation(mband_bf[:], mband_row[:], Act.Identity)
                nc.gpsimd.partition_broadcast(mband_bc[:], mband_bf[:])
                # fp8 feature staging issued early: the scalar-engine fgbp
                # convert gates the whole vector box-filter chain
                fgbp = prep.tile([NCH, W, 68], BF, tag="fgbp")
                nc.gpsimd.memset(fgbp[:], 0.0)
                nc.scalar.activation(fgbp[:, :, 2:66], fg_sb[:], Act.Identity)
                bgbandp = prep.tile([NCH, 18, 66], F32, tag="bgbandp")
                nc.gpsimd.memset(bgbandp[:], 0.0)
                nc.vector.tensor_mul(
                    bgbandp[:, :, 1:65], fgband_sb[:],
                    mband_bc[:].rearrange("c (r x) -> c r x", r=18))
                kernT = prep.tile([NCH, 9, LS], BF, tag="kernT")
                for d in range(9):
                    dy, dx = d // 3, d % 3
                    nc.vector.tensor_scalar_add(
                        kernT[:, d, :],
                        bgbandp[:, dy:dy + 16, dx:dx + 64], EPS)
                for d in range(9):
                    nc.scalar.activation(kern8[:, d, :], kernT[:, d, :],
                                         Act.Identity)

                # ---- kernel norms: sumsq via squares + ones-matmul ----
                onesb = prep.tile([128, 1], BF, tag="onesb")
                nc.gpsimd.memset(onesb[:], 1.0)
                ksq = prep.tile([NCH, LS], BF, tag="ksq")
                ps_s0 = psS.tile([1, 512], F32, tag="psS")
                ps_s1 = psS.tile([1, 512], F32, tag="psS")
                for d in range(9):
                    nc.vector.tensor_mul(ksq[:], kernT[:, d, :],
                                         kernT[:, d, :])
                    nc.tensor.matmul(ps_s0[:], onesb[:], ksq[:, 0:512],
                                     start=(d == 0), stop=(d == 8))
                    nc.tensor.matmul(ps_s1[:], onesb[:], ksq[:, 512:1024],
                                     start=(d == 0), stop=(d == 8))
                srow = prep.tile([1, LS], F32, tag="srow")
                nc.scalar.activation(srow[:, 0:512], ps_s0[:], Act.Identity)
                nc.scalar.activation(srow[:, 512:1024], ps_s1[:],
                                     Act.Identity)
                nc.gpsimd.dma_start(nrm_dram[:], srow[:])
                sq128 = prep.tile([128, LT], F32, tag="sq128")
                nc.gpsimd.dma_start(
                    sq128[:],
                    nrm_dram[:].rearrange("(t p) -> p t", p=128))
                norm128 = prep.tile([128, LT], F32, tag="norm128")
                nc.scalar.activation(norm128[:], sq128[:], Act.Sqrt)

                # ---- box filter via zero-padded shifts, then spaced fp8 --
                t1 = prep.tile([NCH, W, 66], BF, tag="t1")
                nc.vector.tensor_add(t1[:], fgbp[:, :, 0:66],
                                     fgbp[:, :, 1:67])
                tmpHp = prep.tile([NCH, 68, 66], BF, tag="tmpHp")
                nc.gpsimd.memset(tmpHp[:], 0.0)
                nc.vector.tensor_add(tmpHp[:, 2:66, :], t1[:],
                                     fgbp[:, :, 2:68])
                t2 = prep.tile([NCH, 66, 66], BF, tag="t1")
                nc.vector.tensor_add(t2[:], tmpHp[:, 0:66, :],
                                     tmpHp[:, 1:67, :])
                boxbf = prep.tile([NCH, 66, 66], BF, tag="boxbf")
                nc.vector.tensor_add(boxbf[:], t2[:], tmpHp[:, 2:68, :])
                nc.gpsimd.memset(box_sp[:, 0:2], 0.0)
                nc.scalar.activation(
                    rap(box_b, 2, [[ROWB, 66], [2, 66]]), boxbf[:],
                    Act.Identity)
                # reciprocal last on the vector queue so the box filter is
                # not blocked behind the norm-flatten DMA latency
                nc.vector.reciprocal(rnorm_col[:], norm128[:])

                # ---- blend constant: q32 = (1-mask)/9 in stat layout ----
                nc.vector.tensor_scalar(q32[:], m32a[:], -1.0 / 9.0,
                                        1.0 / 9.0, op0=Alu.mult, op1=Alu.add)

                # ---- attn-map border zeros (interior is overwritten) ----
                # row y=-1 and y=64 planes, plus the wrap slots (x'=-1 of
                # every row == x'=64 of the row above) and the lead pad
                nc.gpsimd.memset(rap(ah_b, 2, [[PL, LT], [1, ROWB]]), 0.0)
                nc.gpsimd.memset(
                    rap(ah_b, 2 + 65 * ROWB, [[PL, LT], [1, ROWB]]), 0.0)
                # bytes h*132..h*132+3 = x'=64 slot of row h-1 (+0) and
                # x'=-1 slot of row h (+2), for every row incl. lead pad
                nc.gpsimd.memset(
                    rap(ah_b, 0, [[PL, LT], [ROWB, 67], [1, 4]]), 0.0)

                # ---- kern_lc[l, d, c] via PE transposes, rnorm folded ----
                for d in range(9):
                    for t in range(LT):
                        pt = psA.tile([128, 256], BF, tag="psA")
                        nc.tensor.transpose(
                            pt[:, 0:128],
                            kernT[:, d, t * 128:(t + 1) * 128], ident_b[:])
                        nc.scalar.activation(
                            kern_lc[:, d, t, :], pt[:, 0:128], Act.Identity,
                            scale=rnorm_col[:, t:t + 1])

            # ---------------- chunk-loop pools ----------------
            ctx_scs = tc.tile_pool(name="scs", bufs=2)
            scsp = ctx_scs.__enter__()
            ctx_st = tc.tile_pool(name="stat", bufs=2)
            st = ctx_st.__enter__()
            ctx_bl = tc.tile_pool(name="blend", bufs=2)
            bl = ctx_bl.__enter__()

            def ps_win(ps, h):
                """[128, 4, 64] view of the half-window outputs in psum."""
                return ps[:, 1:265].rearrange(
                    "p (r x) -> p r x", x=66)[:, :, 0:64]

            def emit_gemm1(k):
                """scores for chunk k -> scs (normalized) + mtmp (max)."""
                r0 = k * ROWS
                scs = scsp.tile([128, LT, CS], F32, tag="scs")
                mtmp = st.tile([128, CS], F32, tag="mtmp")
                for t in range(LT):
                    ts = slice(t * 128, (t + 1) * 128)
                    for h in range(2):
                        ps = psA.tile([128, 512], F32, tag="psA")
                        for i, (d0, d1) in enumerate(D_PAIRS):
                            dy, dx = d0 // 3, d0 % 3
                            o = (r0 + 4 * h + dy) * ROWB + 2 * dx
                            nc.tensor.matmul(
                                ps[:, 0:NSTR], kern8[:, d0:d0 + 2, ts],
                                rap(box_b, o,
                                    [[D_DELTA[i], 2], [2, NSTR]]),
                                start=(i == 0), stop=False, perf_mode=DRow)
                        o8 = (r0 + 4 * h + 2) * ROWB + 4
                        nc.tensor.matmul(
                            ps[:, 0:NSTR], kern8[:, 8, ts],
                            box_sp[:, o8:o8 + 2 * NSTR:2],
                            start=False, stop=True)
                        nc.scalar.activation(
                            scs[:, t, :].rearrange(
                                "p (r x) -> p r x", r=ROWS)[:, 4 * h:4 * h + 4, :],
                            ps_win(ps, h), Act.Identity,
                            scale=rnorm_col[:, t:t + 1])
                    if t == 0:
                        nc.vector.tensor_copy(mtmp[:], scs[:, 0, :])
                    else:
                        nc.vector.scalar_tensor_tensor(
                            mtmp[:], scs[:, t, :], 1.0, mtmp[:],
                            op0=Alu.mult, op1=Alu.max)
                return scs, mtmp

            def emit_maxpath(k, mtmp):
                m_bc = st.tile([128, CS], F32, tag="mbc")
                nc.gpsimd.partition_all_reduce(m_bc[:], mtmp[:], 128, RMax)
                return m_bc

            def ah_int(k, t):
                """interior attn window of chunk k, tile t (strided)."""
                r0 = k * ROWS
                return rap(ah_b, t * PL + (1 + r0) * ROWB + 4,
                           [[ROWB, ROWS], [2, 64]])

            def emit_subexp(k, scs, m_bc):
                for t in range(LT):
                    diff = st.tile([128, CS], F32, tag="diff")
                    nc.vector.tensor_sub(diff[:], scs[:, t, :], m_bc[:])
                    nc.scalar.activation(
                        ah_int(k, t),
                        diff[:].rearrange("p (r x) -> p r x", r=ROWS),
                        Act.Exp)

            def emit_sums(k, ag_in, m_bc):
                slot = k % 2
                ps_sum = psS.tile([1, CS], F32, tag="psS")
                for t in range(LT):
                    nc.tensor.matmul(
                        ps_sum[:], ones_s[:], ah_int(k, t),
                        start=(t == 0), stop=(t == LT - 1))
                s_row = st.tile([1, CS], F32, tag="srowc")
                nc.scalar.activation(s_row[:], ps_sum[:], Act.Identity)
                nc.gpsimd.dma_start(
                    ag_in[slot * 2 * CS:slot * 2 * CS + CS], m_bc[0:1, :])
                nc.gpsimd.dma_start(
                    ag_in[slot * 2 * CS + CS:(slot + 1) * 2 * CS], s_row[:])

            def emit_ag(pair):
                ag_out = dram.tile([4 * CS * G], F32, tag="ago")
                nc.gpsimd.collective_compute(
                    "AllGather", Alu.bypass, replica_groups=groups,
                    ins=[pair["in"].opt()], outs=[ag_out.opt()])
                pair["out"] = ag_out

            def emit_combine(k, pair):
                """gathered stats -> w_bc = fac * (1-mask)/9 broadcast.
                [32, 16] stat layout (linear col order, like the dumps)."""
                slot = k % 2
                ag_in, ag_out = pair["in"], pair["out"]
                cm = st.tile([32, G, CS // 32], F32, tag="cm")
                cs = st.tile([32, G, CS // 32], F32, tag="cs")
                for r in range(G):
                    base = r * 4 * CS + slot * 2 * CS
                    nc.gpsimd.dma_start(cm[:, r, :], ag_out[base:base + CS])
                    nc.gpsimd.dma_start(cs[:, r, :],
                                        ag_out[base + CS:base + 2 * CS])
                m32 = st.tile([32, CS // 32], F32, tag="m32")
                nc.gpsimd.dma_start(
                    m32[:], ag_in[slot * 2 * CS:slot * 2 * CS + CS])
                Mx = st.tile([32, CS // 32], F32, tag="Mx")
                nc.vector.tensor_reduce(
                    Mx[:], cm[:].rearrange("p r t -> p t r"), AxX, Alu.max)
                for r in range(G):
                    nc.vector.tensor_sub(cm[:, r, :], cm[:, r, :], Mx[:])
                nc.scalar.activation(cm[:], cm[:], Act.Exp)
                nc.vector.tensor_mul(cs[:], cs[:], cm[:])
                gs = st.tile([32, CS // 32], F32, tag="gs")
                nc.vector.tensor_reduce(
                    gs[:], cs[:].rearrange("p r t -> p t r"), AxX, Alu.add)
                rg = st.tile([32, CS // 32], F32, tag="rg")
                nc.vector.reciprocal(rg[:], gs[:])
                w_sl = st.tile([32, CS // 32], F32, tag="wsl")
                nc.vector.tensor_sub(w_sl[:], m32[:], Mx[:])
                nc.scalar.activation(w_sl[:], w_sl[:], Act.Exp)
                nc.vector.tensor_mul(w_sl[:], w_sl[:], rg[:])
                nc.vector.tensor_mul(w_sl[:], w_sl[:], q32[:, k, :])
                w_dram = dram.tile([CS], F32, tag="wd")
                nc.gpsimd.dma_start(w_dram[:], w_sl[:])
                w_row = st.tile([1, CS], F32, tag="wrow")
                nc.gpsimd.dma_start(w_row[:], w_dram[:])
                w_bc = st.tile([128, CS], F32, tag="wbc")
                nc.gpsimd.partition_broadcast(w_bc[:], w_row[:])
                return w_bc

            def emit_blend_prefetch(k):
                mrowk = bl.tile([1, CS], F32, tag="mrowk")
                nc.sync.dma_start(mrowk[:], mask_ext[:, k * CS:(k + 1) * CS])
                mbc = bl.tile([128, CS], F32, tag="mbck")
                nc.gpsimd.partition_broadcast(mbc[:], mrowk[:])
                fgc = bl.tile([NCH, CS], F32, tag="fgc")
                nc.sync.dma_start(fgc[:], fg_ext[:, k * CS:(k + 1) * CS])
                return mbc, fgc

            def emit_gemm2(k):
                r0 = k * ROWS
                out = []
                for h in range(2):
                    ps2 = psB.tile([128, 512], F32, tag="psB")
                    n = 0
                    for dyp in range(3):
                        for dxp in range(3):
                            dflip = (2 - dyp) * 3 + (2 - dxp)
                            o = (r0 + 4 * h + dyp) * ROWB + 2 * dxp
                            for tp in range(LT // 2):
                                nc.tensor.matmul(
                                    ps2[:, 0:NSTR],
                                    kern_lc[:, dflip, 2 * tp:2 * tp + 2, :],
                                    rap(ah_b, 2 * tp * PL + o,
                                        [[PL, 2], [2, NSTR]]),
                                    start=(n == 0), stop=(n == 35),
                                    perf_mode=DRow)
                                n += 1
                    out.append(ps2)
                return out

            def emit_blend(k, ps2s, w_bc, mbc, fgc, rs_in, slot):
                mfk = bl.tile([128, CS], F32, tag="mfk")
                nc.vector.scalar_tensor_tensor(
                    mfk[:], fgc[:], 1.0 / G, mbc[:], op0=Alu.mult,
                    op1=Alu.mult)
                out_sb = bl.tile([128, CS], F32, tag="outsb")
                ov = out_sb[:].rearrange("p (r x) -> p r x", r=ROWS)
                wv = w_bc[:].rearrange("p (r x) -> p r x", r=ROWS)
                for h in range(2):
                    nc.vector.tensor_mul(ov[:, 4 * h:4 * h + 4, :],
                                         ps_win(ps2s[h], h),
                                         wv[:, 4 * h:4 * h + 4, :])
                nc.vector.tensor_add(out_sb[:], out_sb[:], mfk[:])
                nc.sync.dma_start(
                    rs_in[:, slot * CS:(slot + 1) * CS], out_sb[:])

            def emit_rs(j0, nch, rs_in):
                rs_out = dram.tile([NCH // G, nch * CS], F32, tag="rso")
                nc.gpsimd.collective_compute(
                    "ReduceScatter", Alu.add, replica_groups=groups,
                    ins=[rs_in.opt()], outs=[rs_out.opt()])
                nc.sync.dma_start(
                    out_ext[:, j0 * CS:(j0 + nch) * CS], rs_out[:])

            # ---------------- software pipeline ----------------
            g1 = {0: emit_gemm1(0)}
            mp = {0: emit_maxpath(0, g1[0][1])}
            ag_pairs = {}
            rs_bufs = {}
            ps2s = {}
            blf = {}
            for k in range(NCHUNK + 2):
                jbs = [j for j in
                       {NCHUNK: [NCHUNK - 3, NCHUNK - 2],
                        NCHUNK + 1: [NCHUNK - 1]}.get(k, [k - 3])
                       if 0 <= j <= NCHUNK - 1]
                wbs = {j: emit_combine(j, ag_pairs[j // 2]) for j in jbs}
                if 0 <= k - 2 <= NCHUNK - 1:
                    blf[k - 2] = emit_blend_prefetch(k - 2)
                    ps2s[k - 2] = emit_gemm2(k - 2)
                for j in jbs:
                    mbc, fgc = blf.pop(j)
                    if j < NCHUNK - 2:
                        emit_blend(j, ps2s.pop(j), wbs[j], mbc, fgc,
                                   rs_bufs[j // 2], j % 2)
                        if j % 2 == 1:
                            emit_rs(j - 1, 2, rs_bufs.pop(j // 2))
                    else:
                        # last two chunks scatter individually (short tail)
                        emit_blend(j, ps2s.pop(j), wbs[j], mbc, fgc,
                                   rs_bufs[j], 0)
                        emit_rs(j, 1, rs_bufs.pop(j))
                if k <= NCHUNK - 1:
                    if k % 2 == 0:
                        ag_pairs[k // 2] = {
                            "in": dram.tile([4 * CS], F32, tag="agi",
                                            name=f"agi{k // 2}")}
                        if k < NCHUNK - 2:
                            rs_bufs[k // 2] = dram.tile(
                                [NCH, 2 * CS], F32, tag="rsi",
                                name=f"rsi{k // 2}")
                        else:
                            rs_bufs[k] = dram.tile(
                                [NCH, CS], F32, tag="rsj", name=f"rsj{k}")
                            rs_bufs[k + 1] = dram.tile(
                                [NCH, CS], F32, tag="rsj", name=f"rsj{k + 1}")
                    emit_subexp(k, g1[k][0], mp[k])
                if k + 1 <= NCHUNK - 1:
                    g1[k + 1] = emit_gemm1(k + 1)
                    mp[k + 1] = emit_maxpath(k + 1, g1[k + 1][1])
                if k <= NCHUNK - 1:
                    emit_sums(k, ag_pairs[k // 2]["in"], mp.pop(k))
                    g1.pop(k)
                    if k % 2 == 1:
                        emit_ag(ag_pairs[k // 2])

            ctx_bl.__exit__(None, None, None)
            ctx_st.__exit__(None, None, None)
            ctx_scs.__exit__(None, None, None)

    nc.compile()
    return nc


def _shard_inputs(fg, mk):
    """fg [2,128,64,64] f32, mk [2,1,64,64] f32 -> per-core input maps."""
    in_maps = []
    for core in range(NCORES):
        b, r = core // G, core % G
        y0 = r * (W // G)
        feat = np.ascontiguousarray(fg[b].reshape(NCH, S), np.float32)
        mask = np.ascontiguousarray(mk[b].reshape(1, S), np.float32)
        band = np.zeros((NCH, 18, H), np.float32)
        mband = np.zeros((1, 18, H), np.float32)
        lo = y0 - 1
        src_lo = max(0, lo)
        src_hi = min(W, y0 + 17)
        band[:, src_lo - lo:src_hi - lo] = fg[b][:, src_lo:src_hi]
        mband[:, src_lo - lo:src_hi - lo] = mk[b][:, src_lo:src_hi]
        in_maps.append({
            "fg": feat,
            "fgband": np.ascontiguousarray(band.reshape(NCH, 18 * H)),
            "mask": mask,
            "maskband": np.ascontiguousarray(mband.reshape(1, 18 * H)),
        })
    return in_maps


def kernel(foreground, masks):
    global LAST_EXEC_TIME_NS
    from concourse.bass_utils import run_bass_kernel_spmd

    fg = np.asarray(foreground, np.float32)
    mk = np.asarray(masks, np.float32)
    assert fg.shape == (B, NCH, W, H) and mk.shape == (B, 1, W, H)

    nc = _CACHE.get("nc")
    if nc is None:
        nc = _build()
        _CACHE["nc"] = nc

    in_maps = _shard_inputs(fg, mk)
    trace = bool(os.environ.get("BASS_KERNEL_TRACE"))
    res = run_bass_kernel_spmd(nc, in_maps, core_ids=list(range(NCORES)),
                               trace=trace)
    LAST_EXEC_TIME_NS = res.exec_time_ns
    if res.exec_time_ns is not None:
        print(f"HW exec time: {res.exec_time_ns} ns")

    out = np.empty((B, NCH, W, H), np.float32)
    for core in range(NCORES):
        b, r = core // G, core % G
        out[b, 32 * r:32 * (r + 1)] = (
            res.results[core]["out"].reshape(32, W, H))
    return out
